# revision 2
# baseline (speedup 1.0000x reference)
"""BertBiLSTMCRF loss kernel for 8 Trainium2 NeuronCores.

Sharding: data-parallel over batch (B=32 -> 4 sentences/core). The BERT
encoder (>95% of FLOPs) runs on-device in raw Bass. Activations are kept
in transposed layout hT=[H, tokens] on chip so every GEMM consumes
weights in their stored [in,out] layout as lhsT with no activation
transposes; attention computes S^T (k on partitions), uses unnormalized
exp (scores are tiny after LN + 0.02-scale weights) and gets the softmax
denominator via a ones-column matmul, so no partition-dim max/sum is
ever needed. The BiLSTM/CRF tail (small FLOPs, serial scans) runs on
host, as does the embedding gather.
"""
import os
import numpy as np
from scipy.special import erf

V, H, NL, NH, S, B, HL, T = 30522, 768, 12, 12, 256, 32, 256, 9
DH = H // NH
FF = 4 * H
NCORES = 8
BL = B // NCORES          # sentences per core
TOK = BL * S              # tokens per core (1024)
KT = H // 128             # 6 k-tiles over hidden
MT_TOK = TOK // 128       # 8 token m-tiles
FP32R = os.environ.get("KERNEL_NO_FP32R", "") == ""
USE_DEVICE = os.environ.get("KERNEL_HOST", "") == ""
DEV_LAYERS = int(os.environ.get("KERNEL_LAYERS", str(NL)))

LAST_HW_NS = None
_CACHE = {}


# ---------------------------------------------------------------- host math
def _ln_np(x, g, b):
    m = x.mean(-1, keepdims=True)
    v = ((x - m) ** 2).mean(-1, keepdims=True)
    return (x - m) / np.sqrt(v + 1e-12) * g + b


def _gelu_np(x):
    return (0.5 * x * (1.0 + erf(x / np.float32(np.sqrt(2.0))))).astype(np.float32)


def _sigmoid_np(x):
    return 1.0 / (1.0 + np.exp(-x))


def _bert_host(h, a, n_layers=NL):
    Bc = h.shape[0]
    for l in range(n_layers):
        qkv = h @ a['Wqkv'][l] + a['bqkv'][l]
        q, k, v = [t.reshape(Bc, S, NH, DH) for t in np.split(qkv, 3, axis=-1)]
        sc = np.einsum('bqhd,bkhd->bhqk', q, k) / np.float32(np.sqrt(DH))
        sc = sc - sc.max(-1, keepdims=True)
        p = np.exp(sc)
        p = p / p.sum(-1, keepdims=True)
        ctx = np.einsum('bhqk,bkhd->bqhd', p, v).reshape(Bc, S, H)
        h = _ln_np(h + ctx @ a['Wo'][l] + a['bo'][l], a['ln1_g'][l], a['ln1_b'][l])
        ff = _gelu_np(h @ a['W1'][l] + a['b1'][l]) @ a['W2'][l] + a['b2'][l]
        h = _ln_np(h + ff, a['ln2_g'][l], a['ln2_b'][l])
    return h


def _lstm_host(x, Wih, Whh, bih, bhh, reverse):
    Bc = x.shape[0]
    pre = np.swapaxes(x, 0, 1) @ Wih.T + (bih + bhh)  # [S,B,4H]
    hs = np.zeros((S, Bc, HL), np.float32)
    h = np.zeros((Bc, HL), np.float32)
    c = np.zeros((Bc, HL), np.float32)
    order = range(S - 1, -1, -1) if reverse else range(S)
    for t in order:
        g = pre[t] + h @ Whh.T
        i, f, gg, o = np.split(g, 4, axis=-1)
        c = _sigmoid_np(f) * c + _sigmoid_np(i) * np.tanh(gg)
        h = _sigmoid_np(o) * np.tanh(c)
        hs[t] = h
    return np.swapaxes(hs, 0, 1)


def _logsumexp(a, axis):
    m = a.max(axis=axis, keepdims=True)
    return (np.log(np.exp(a - m).sum(axis=axis, keepdims=True)) + m).squeeze(axis)


def _crf_host(logits, labels, maskf, crf_start, crf_end, crf_trans):
    em = np.take_along_axis(logits, labels[..., None], -1)[..., 0]
    tr = crf_trans[labels[:, :-1], labels[:, 1:]]
    last_idx = maskf.sum(1).astype(np.int32) - 1
    last_tag = np.take_along_axis(labels, last_idx[:, None], 1)[:, 0]
    num = (crf_start[labels[:, 0]] + em[:, 0]
           + ((em[:, 1:] + tr) * maskf[:, 1:]).sum(1) + crf_end[last_tag])
    alpha = crf_start + logits[:, 0]
    for t in range(1, S):
        nxt = _logsumexp(alpha[:, :, None] + crf_trans[None] + logits[:, t][:, None, :], 1)
        alpha = np.where(maskf[:, t][:, None] > 0, nxt, alpha)
    den = _logsumexp(alpha + crf_end, -1)
    return den - num


# ------------------------------------------------------------ device program
class Prog:
    """Raw-Bass multi-engine program recorder with conservative sync:
    each op waits until everything its producer engines emitted so far is
    done. Duplicate waits are elided per consumer engine. The DMA
    semaphore rotates per layer to stay far from counter limits."""

    def __init__(self):
        self.ops = {e: [] for e in ("pe", "act", "dve", "dma")}
        self.counts = {}              # sem name -> emitted count
        self.seen = {e: {} for e in self.ops}
        self.cur_dma = "dmaS0"
        self.sem_names = {"pe", "act", "dve", "dmaS0"}

    def next_dma_sem(self, name):
        self.cur_dma = name
        self.sem_names.add(name)

    def _resolve(self, dep):
        if dep == "dma":
            return [s for s in self.sem_names if s.startswith("dmaS")]
        return [dep]

    def emit(self, engine, fn, deps=()):
        waits = []
        for d in deps:
            for sem in self._resolve(d):
                if sem == engine:
                    continue
                val = self.counts.get(sem, 0)
                if val > 0 and self.seen[engine].get(sem, -1) < val:
                    waits.append((sem, val))
                    self.seen[engine][sem] = val
        sem_self = self.cur_dma if engine == "dma" else engine
        inc = 16 if engine == "dma" else 1
        self.counts[sem_self] = self.counts.get(sem_self, 0) + inc
        self.ops[engine].append((waits, fn, sem_self, inc))

    def replay(self, engine, eng, sems):
        for waits, fn, sem_self, inc in self.ops[engine]:
            for name, val in waits:
                eng.wait_ge(sems[name], val)
            fn().then_inc(sems[sem_self], inc)


def _build_encoder(n_layers):
    import concourse.bass as bass
    import concourse.mybir as mybir
    from contextlib import ExitStack
    dt = mybir.dt
    f32 = dt.float32
    AF = mybir.ActivationFunctionType
    ALU = mybir.AluOpType

    nc = bass.Bass()
    ctx = ExitStack()

    def mmdt(ap):
        return ap.bitcast(dt.float32r) if FP32R else ap

    def R(ap):
        # round-on-write for tiles later consumed by fp32r matmuls
        return ap.bitcast(dt.float32r) if FP32R else ap

    # ---- DRAM parameters
    hT0 = nc.declare_dram_parameter("hT0", [H, TOK], f32, isOutput=False)
    Wqkv = nc.declare_dram_parameter("Wqkv", [NL, H, 3 * H], f32, isOutput=False)
    Wo = nc.declare_dram_parameter("Wo", [NL, H, H], f32, isOutput=False)
    W1 = nc.declare_dram_parameter("W1", [NL, H, FF], f32, isOutput=False)
    W2 = nc.declare_dram_parameter("W2", [NL, FF, H], f32, isOutput=False)
    biasall = nc.declare_dram_parameter("biasall", [NL, 128, 80], f32, isOutput=False)
    consts = nc.declare_dram_parameter("consts", [128, 1024], f32, isOutput=False)
    onesd = nc.declare_dram_parameter("onesd", [128, 1], f32, isOutput=False)
    hTout = nc.declare_dram_parameter("hTout", [H, TOK], f32, isOutput=True)
    zscr = nc.dram_tensor("zscr", [4, 3072], f32)

    # ---- on-chip tensors
    sbt = lambda nm, shape: ctx.enter_context(nc.sbuf_tensor(nm, shape, f32))
    hT = sbt("hT", [128, KT, TOK])
    h1T = sbt("h1T", [128, KT, TOK])
    ctxT = sbt("ctxT", [128, KT * TOK])   # flat; viewed [128, 6, 1024]
    big = sbt("bigb", [128, 12, TOK])     # qkT in attn; ff1 tiles 0-11; LN sq
    vbuf = sbt("vbuf", [128, KT * TOK])   # flat; v=[128,8,768] / ff1 18-23
    wsl = sbt("wsl", [128, 2, 3072])      # weight slab, 2 slots
    bias = sbt("biassb", [128, 80])
    csts = sbt("csts", [128, 904])
    stats = sbt("stats", [1, 2048])   # col blocks: mean | E2/var/istd
    stats2 = sbt("stats2", [1, 1024])  # istd
    zbuf4 = sbt("zbuf4", [97, 3072])   # Z at partition bases 0/32/64/96
    zbuf = sbt("zbuf", [12, 1024])     # Z reshaped for broadcast matmul
    expS = sbt("expS", [128, 2, S])
    onesr = sbt("onesr", [128, 1])

    psA = ctx.enter_context(nc.psum_tensor("psA", [128, 1024], f32))
    psB = ctx.enter_context(nc.psum_tensor("psB", [128, 1024], f32))
    psS = ctx.enter_context(nc.psum_tensor("psS", [128, 2, S], f32))
    psC = ctx.enter_context(nc.psum_tensor("psC", [128, S], f32))

    ctxTv = ctxT[:, :].rearrange("p (n t) -> p n t", t=TOK)

    def vtile(m):                     # v token-tile m: [128, 768]
        return vbuf[:, m * H:(m + 1) * H]

    def fftile(kt):                   # ff1 feature k-tile: [128, 1024]
        if kt < 12:
            return big[:, kt, :]
        if kt < 18:
            return ctxTv[:, kt - 12, :]
        return vbuf[:, (kt - 18) * TOK:(kt - 17) * TOK]

    P = Prog()
    CD = ("pe", "act", "dve", "dma")

    DMA_FULL_SYNC = os.environ.get("KERNEL_DMA_FULL_SYNC", "") != ""

    def dma(dst, src, deps=("pe", "act", "dve")):
        if DMA_FULL_SYNC:
            deps = CD
        P.emit("dma", lambda d=dst, s=src: nc.sync.dma_start(out=d, in_=s),
               deps=deps)

    def mm(out, lhsT, rhs, start, stop, raw=False):
        if raw:
            P.emit("pe", lambda o=out, l=lhsT, r=rhs, a=start, b=stop:
                   nc.tensor.matmul(o, l, r, start=a, stop=b), deps=CD)
        else:
            P.emit("pe", lambda o=out, l=lhsT, r=rhs, a=start, b=stop:
                   nc.tensor.matmul(o, mmdt(l), mmdt(r), start=a, stop=b),
                   deps=CD)

    def act(out, in_, func, b=0.0, scale=1.0):
        P.emit("act", lambda o=out, i=in_, f=func, bb=b, s=scale:
               nc.scalar.activation(o, i, f, bias=bb, scale=s), deps=CD)

    def dve_tt(out, in0, in1, op):
        P.emit("dve", lambda o=out, x=in0, y=in1, z=op:
               nc.vector.tensor_tensor(o, x, y, z), deps=CD)

    def dve_ts(out, in_, s1, s2, op0, op1):
        P.emit("dve", lambda o=out, i=in_, a=s1, b=s2, x=op0, y=op1:
               nc.vector.tensor_scalar(o, i, a, b, x, y), deps=CD)

    def dve_recip(out, in_):
        P.emit("dve", lambda o=out, i=in_: nc.vector.reciprocal(o, i), deps=CD)

    # ---- boot: constants + initial activations
    dma(csts[:, :], consts[:, 0:904], deps=())
    dma(R(onesr[:, :]), R(onesd[:, :]), deps=())
    dma(R(hT[:, :, :]), R(hT0.rearrange("(n p) t -> p n t", p=128)), deps=())
    ones128 = onesr[:, 0:1]
    onesrow = csts[0:1, 2:130]        # [1,128] ones on partition 0

    def m12(m):                       # [12, 128] head-broadcast map k-tile
        return csts[0:12, 130 + m * 128:130 + (m + 1) * 128]

    def stream_gemm(W_dram, n_in, n_out, rhs_tile_fn, out_fn, bias_fn,
                    act_fn):
        """out[m] = act(sum_kt W[kt,m].T @ rhs[kt] + bias[m]); W streamed
        through wsl slots (per m-tile)."""
        kt_n = n_in // 128
        mt_n = n_out // 128
        for m in range(mt_n):
            slot = wsl[:, m % 2, :]
            for kt in range(kt_n):
                dma(R(slot[:, kt * 128:(kt + 1) * 128]),
                    R(W_dram[kt * 128:(kt + 1) * 128, m * 128:(m + 1) * 128]),
                    deps=("pe",))
            for half in range(2):
                ps = psA[:, half * 512:(half + 1) * 512]
                for kt in range(kt_n):
                    mm(ps, slot[:, kt * 128:(kt + 1) * 128],
                       rhs_tile_fn(kt)[:, half * 512:(half + 1) * 512],
                       start=(kt == 0), stop=(kt == kt_n - 1))
            act(R(out_fn(m)), psA[:, :TOK], act_fn, b=bias_fn(m))

    def layernorm(x, gcol0, bcol0, gbuf, sq):
        # x: [128, KT, TOK] feature-major; returns in place
        for kt in range(KT):
            act(R(sq[:, kt, :]), x[:, kt, :], AF.Square)
        for half in range(2):
            c0, c1 = half * 512, (half + 1) * 512
            for kt in range(KT):
                mm(psA[0:1, c0:c1], ones128, x[:, kt, c0:c1],
                   start=(kt == 0), stop=(kt == KT - 1))
            for kt in range(KT):
                mm(psB[0:1, c0:c1], ones128, sq[:, kt, c0:c1],
                   start=(kt == 0), stop=(kt == KT - 1))
        mean = stats[0:1, 0:1024]
        blk = stats[0:1, 1024:2048]       # E2 -> var -> istd, in place
        tmp = stats2[0:1, :]              # meansq -> sd
        act(mean, psA[0:1, :], AF.Identity, scale=1.0 / H)
        act(blk, psB[0:1, :], AF.Identity, scale=1.0 / H)
        dve_tt(tmp, mean, mean, ALU.mult)
        dve_tt(blk, blk, tmp, ALU.subtract)
        P.emit("dve", lambda: nc.vector.tensor_scalar_add(blk, blk, 1e-12),
               deps=CD)
        act(tmp, blk, AF.Sqrt)
        dve_recip(blk, tmp)                                           # istd
        for half in range(2):
            c0, c1 = half * 512, (half + 1) * 512
            mm(psA[:, c0:c1], onesrow, stats[0:1, c0:c1], start=True,
               stop=True, raw=True)
            mm(psB[:, c0:c1], onesrow, stats[0:1, 1024 + c0:1024 + c1],
               start=True, stop=True, raw=True)
        for kt in range(KT):
            dve_tt(R(x[:, kt, :]), x[:, kt, :], psA[:, :TOK], ALU.subtract)
            dve_tt(R(x[:, kt, :]), x[:, kt, :], psB[:, :TOK], ALU.mult)
            dve_ts(R(x[:, kt, :]), x[:, kt, :],
                   gbuf[:, gcol0 + kt:gcol0 + kt + 1],
                   gbuf[:, bcol0 + kt:bcol0 + kt + 1], ALU.mult, ALU.add)

    for l in range(n_layers):
        P.next_dma_sem(f"dmaS{l + 1}")
        dma(bias[:, :], biasall[l])

        # qkT into big[:, 0:12]: features q(0-5) k(6-11)
        stream_gemm(Wqkv[l][:, 0:1536], H, 1536, lambda kt: hT[:, kt, :],
                    lambda m: big[:, m, :], lambda m: bias[:, m:m + 1],
                    AF.Identity)

        # v = hT.T @ Wv  (token-major; bias folded in after softmax)
        for kt in range(KT):
            dma(R(wsl[:, kt % 2, (kt // 2) * 768:(kt // 2) * 768 + 768]),
                R(Wqkv[l][kt * 128:(kt + 1) * 128, 1536:2304]), deps=("pe",))
        for m in range(MT_TOK):
            for c0, c1 in ((0, 512), (512, 768)):
                ps = psA[:, c0:c1]
                for kt in range(KT):
                    wv = wsl[:, kt % 2, (kt // 2) * 768:(kt // 2) * 768 + 768]
                    mm(ps, hT[:, kt, m * 128:(m + 1) * 128], wv[:, c0:c1],
                       start=(kt == 0), stop=(kt == KT - 1))
            act(R(vtile(m)), psA[:, 0:H], AF.Identity)

        # attention
        for s in range(BL):
            for hh in range(NH):
                prow = 64 * (hh % 2)
                qt = big[prow:prow + 64, hh // 2, s * S:(s + 1) * S]
                ktap = big[prow:prow + 64, 6 + hh // 2, s * S:(s + 1) * S]
                for i in range(2):
                    mm(psS[:, i, :], ktap[:, i * 128:(i + 1) * 128], qt,
                       start=True, stop=True)
                act(R(expS[:, :, :]), psS[:, :, :], AF.Exp, scale=1.0 / 8.0)
                for i in range(2):
                    mm(psC[0:64, :], vtile(2 * s + i)[:, hh * 64:(hh + 1) * 64],
                       expS[:, i, :], start=(i == 0), stop=(i == 1))
                    mm(psS[0:1, 0, :], ones128, expS[:, i, :],
                       start=(i == 0), stop=(i == 1))
                act(R(ctxTv[prow:prow + 64, hh // 2, s * S:(s + 1) * S]),
                    psC[0:64, :], AF.Identity)
                zr = zbuf4[32 * (hh % 4):32 * (hh % 4) + 1,
                           (hh // 4) * 1024 + s * S:(hh // 4) * 1024 + (s + 1) * S]
                act(zr, psS[0:1, 0, :], AF.Identity)

        # normalize ctx by Z (per head), add v bias
        for p4 in range(4):
            dve_recip(zbuf4[32 * p4:32 * p4 + 1, :], zbuf4[32 * p4:32 * p4 + 1, :])
        dma(zscr[:, :], zbuf4[0:97:32, :])
        # must wait for the zscr store above: DMAs from one queue are split
        # across 16 SDMA engines with no cross-DMA completion ordering
        dma(zbuf[0:12, :], zscr[:, :].rearrange("p (b t) -> (p b) t", b=3),
            deps=CD)
        for m in range(KT):
            for half in range(2):
                mm(psA[:, half * 512:(half + 1) * 512], m12(m),
                   zbuf[0:12, half * 512:(half + 1) * 512], start=True,
                   stop=True, raw=True)
            dve_tt(R(ctxTv[:, m, :]), ctxTv[:, m, :], psA[:, :TOK], ALU.mult)
            P.emit("dve", lambda m=m: nc.vector.tensor_scalar_add(
                R(ctxTv[:, m, :]), ctxTv[:, m, :], bias[:, 12 + m:13 + m]),
                deps=CD)

        # attn proj + residual + LN1
        stream_gemm(Wo[l], H, H, lambda kt: ctxTv[:, kt, :],
                    lambda m: h1T[:, m, :], lambda m: bias[:, 18 + m:19 + m],
                    AF.Identity)
        for m in range(KT):
            dve_tt(R(h1T[:, m, :]), h1T[:, m, :], hT[:, m, :], ALU.add)
        layernorm(h1T, 24, 30, bias, big[:, 0:KT, :])

        # FF1 (gelu) into big/ctxT/vbuf tiles
        stream_gemm(W1[l], H, FF, lambda kt: h1T[:, kt, :],
                    fftile, lambda m: bias[:, 36 + m:37 + m], AF.Gelu)

        # FF2 + residual + LN2 -> hT
        for m in range(KT):
            slot = wsl[:, m % 2, :]
            for kt in range(24):
                dma(R(slot[:, kt * 128:(kt + 1) * 128]),
                    R(W2[l][kt * 128:(kt + 1) * 128, m * 128:(m + 1) * 128]),
                    deps=("pe",))
            for half in range(2):
                ps = psA[:, half * 512:(half + 1) * 512]
                for kt in range(24):
                    mm(ps, slot[:, kt * 128:(kt + 1) * 128],
                       fftile(kt)[:, half * 512:(half + 1) * 512],
                       start=(kt == 0), stop=(kt == 23))
            act(R(hT[:, m, :]), psA[:, :TOK], AF.Identity, b=bias[:, 60 + m:61 + m])
            dve_tt(R(hT[:, m, :]), hT[:, m, :], h1T[:, m, :], ALU.add)
        layernorm(hT, 66, 72, bias, big[:, 0:KT, :])

    dma(hTout.rearrange("(n p) t -> p n t", p=128), hT[:, :, :])

    # ---- replay into engine blocks
    sems = {}
    for name in sorted(P.sem_names):
        sems[name] = ctx.enter_context(nc.semaphore(name))
    with nc.Block() as block:
        @block.tensor
        def _(eng):
            P.replay("pe", eng, sems)

        @block.scalar
        def _(eng):
            P.replay("act", eng, sems)

        @block.vector
        def _(eng):
            P.replay("dve", eng, sems)

        @block.sync
        def _(eng):
            P.replay("dma", eng, sems)

    return nc, ctx


def _pack_consts():
    c = np.zeros((128, 1024), np.float32)
    c[:, 0] = 1.0                       # ones128
    c[0, 2:130] = 1.0                   # onesrow
    # zbuf row r (after the strided reshape DMA) holds head (r%3)*4 + r//3
    for r in range(NH):
        hh = (r % 3) * 4 + r // 3
        for f in range(H):
            if f // DH == hh:
                c[r, 130 + f] = 1.0
    return c


def _pack_bias(a):
    out = np.zeros((NL, 128, 80), np.float32)

    def col(vec):                       # feature vec [n*128] -> [128, n]
        return vec.reshape(-1, 128).T

    for l in range(NL):
        out[l, :, 0:18] = col(a['bqkv'][l])
        out[l, :, 18:24] = col(a['bo'][l])
        out[l, :, 24:30] = col(a['ln1_g'][l])
        out[l, :, 30:36] = col(a['ln1_b'][l])
        out[l, :, 36:60] = col(a['b1'][l])
        out[l, :, 60:66] = col(a['b2'][l])
        out[l, :, 66:72] = col(a['ln2_g'][l])
        out[l, :, 72:78] = col(a['ln2_b'][l])
    return out


def _profile_ntff(nc, run_fn):
    """Re-run `run_fn` under NRT/NTFF profiling (core 0) and return
    (results, exec_time_ns, trace_path); (results, None, None) if the
    profiling stack is unavailable. neuron-profile measures only the NEFF
    execution on the device, so the returned time is pure HW exec time."""
    import ctypes
    import tempfile

    try:
        lib = ctypes.CDLL("/opt/axon/libaxon_pjrt.so")
        if not hasattr(lib, "axon_start_nrt_profile"):
            return run_fn(), None, None
    except OSError:
        return run_fn(), None, None
    lib.axon_start_nrt_profile.argtypes = [ctypes.POINTER(ctypes.c_int64),
                                           ctypes.c_size_t]
    lib.axon_start_nrt_profile.restype = ctypes.c_int64
    lib.axon_stop_nrt_profile.argtypes = [ctypes.c_char_p]
    lib.axon_stop_nrt_profile.restype = ctypes.c_int64

    import jax
    jax.devices()
    neff_dir = tempfile.mkdtemp(prefix="bassprof_")
    ids = (ctypes.c_int64 * 1)(0)
    if lib.axon_start_nrt_profile(ids, 1) != 0:
        return run_fn(), None, None
    try:
        results = run_fn()
    finally:
        nfiles = lib.axon_stop_nrt_profile(neff_dir.encode())
    if nfiles <= 0:
        return results, None, None
    try:
        from concourse._compat import FishPath
        import gauge.profiler
        profile = gauge.profiler.Profile(
            profile_path=FishPath(neff_dir),
            kernel_dev_mode=True,
            profile_on_exit=False,
            bass_kernel=nc.m,
            offline_processing=True,
            fname="*_body*",
        )
        pres = profile.to_perfetto(model_index=(0,))
        if pres and pres[0].exec_time_ns:
            return results, int(pres[0].exec_time_ns), pres[0].trace_path
    except Exception as e:
        print("[kernel] ntff processing failed:", e)
    return results, None, None


def run_device(h0, a):
    global LAST_HW_NS
    if not USE_DEVICE:
        return _bert_host(h0, a)
    import time
    from concourse.bass_utils import run_bass_kernel_spmd

    key = ("enc", DEV_LAYERS)
    if key not in _CACHE:
        _CACHE[key] = _build_encoder(DEV_LAYERS)
    nc, _ctx = _CACHE[key]

    biasall = _pack_bias(a)
    consts = _pack_consts()
    shared = {"Wqkv": a['Wqkv'], "Wo": a['Wo'], "W1": a['W1'], "W2": a['W2'],
              "biasall": biasall, "consts": consts,
              "onesd": np.ones((128, 1), np.float32)}
    in_maps = []
    for c in range(NCORES):
        hc = h0[c * BL:(c + 1) * BL].reshape(TOK, H).T.copy()  # [H, TOK]
        in_maps.append(dict(shared, hT0=np.ascontiguousarray(hc)))

    cores = list(range(NCORES))
    t0 = time.time()
    res = run_bass_kernel_spmd(nc, in_maps, cores)  # compile + warm run
    warm_wall_ns = int((time.time() - t0) * 1e9)
    LAST_HW_NS = warm_wall_ns
    if getattr(res, "exec_time_ns", None):
        LAST_HW_NS = int(res.exec_time_ns)

    if os.environ.get("KERNEL_NO_PROFILE", "") == "":
        try:
            res2, exec_ns, trace = _profile_ntff(
                nc, lambda: run_bass_kernel_spmd(nc, in_maps, cores))
            if exec_ns:
                res = res2
                LAST_HW_NS = exec_ns
                print("[kernel] profile exec_time_ns:", exec_ns,
                      "trace:", trace)
        except Exception as e:
            print("[kernel] profiling failed, using wall time:", e)

    h = np.zeros((B, S, H), np.float32)
    for c in range(NCORES):
        h[c * BL:(c + 1) * BL] = res.results[c]["hTout"].T.reshape(BL, S, H)
    if DEV_LAYERS < NL:                 # debugging path: finish on host
        a2 = {k: (v[DEV_LAYERS:] if k in ("Wqkv", "bqkv", "Wo", "bo", "ln1_g",
              "ln1_b", "W1", "b1", "W2", "b2", "ln2_g", "ln2_b") else v)
              for k, v in a.items()}
        h = _bert_host(h, a2, NL - DEV_LAYERS)
    return h


def kernel(input_ids, attention_mask, labels, emb_tok, emb_pos, emb_type,
           ln_emb_g, ln_emb_b, Wqkv, bqkv, Wo, bo, ln1_g, ln1_b, W1, b1,
           W2, b2, ln2_g, ln2_b, Wih_f, Whh_f, bih_f, bhh_f, Wih_b, Whh_b,
           bih_b, bhh_b, Wc, bc, tag_weight, crf_start, crf_end, crf_trans):
    args = {k: np.asarray(v) for k, v in locals().items()}
    maskf = args['attention_mask'].astype(np.float32)

    h0 = (args['emb_tok'][args['input_ids']] + args['emb_pos'][:S][None]
          + args['emb_type'][0][None, None]).astype(np.float32)
    h0 = _ln_np(h0, args['ln_emb_g'], args['ln_emb_b'])

    h = run_device(h0, args)

    hf = _lstm_host(h, args['Wih_f'], args['Whh_f'], args['bih_f'], args['bhh_f'], False)
    hb = _lstm_host(h, args['Wih_b'], args['Whh_b'], args['bih_b'], args['bhh_b'], True)
    logits = (np.concatenate([hf, hb], -1) @ args['Wc'] + args['bc']) * args['tag_weight']
    ll = _crf_host(logits, args['labels'], maskf, args['crf_start'],
                   args['crf_end'], args['crf_trans'])
    return np.float32(ll.mean())



# revision 27
# speedup vs baseline: 2.5573x; 2.5573x over previous
"""BertBiLSTMCRF loss kernel for 8 Trainium2 NeuronCores.

Sharding: data-parallel over batch (B=32 -> 4 sentences/core). The BERT
encoder (>95% of FLOPs) runs on-device in raw Bass; embeddings, the
BiLSTM and the CRF (small FLOPs, serial scans) run on host.

Device kernel design (v2):
- Activations and weights are bf16 on chip (fp32 PSUM accumulate, fp32
  LN statistics). Halves DMA traffic and doubles DVE throughput; PE rate
  matches fp32r while numerics stay far inside the 2e-2 budget.
- Activations live transposed, hT = [feature, token], so every GEMM
  consumes weights in stored [in,out] layout as lhsT with no activation
  transposes.
- Weights are host-packed so each m-tile's slab is ONE contiguous DMA,
  streamed through rotating SBUF slots with deep prefetch. Every weight
  slot has its own DMA semaphore so at most one transfer is ever
  outstanding per semaphore -- count waits are then exact even though
  SDMA completions reorder across engines.
- Fine-grained cross-engine sync: ops wait only on snapshot counts of
  the semaphores they actually depend on, with PSUM double-buffering
  (psA/psB) so the PE streams matmuls continuously while ACT/DVE drain
  behind it and DMA prefetches ahead. This also keeps the PE HAM
  clock-gate warm (2.4 GHz) instead of the 1.2 GHz it falls to when the
  PE idles between bursts (the baseline spent 75% of its time there).
- Attention: the two heads of a pair compute scores via concurrent
  row-group matmuls (partition bases 0/64), one exp covers the whole
  pair, and the softmax denominator comes free from a ones column
  appended to v (it lands in row 64 of the ctx psum). Unnormalized exp
  is safe here (LN'd inputs, 0.02-scale weights).
- LayerNorm: mean/sq sums are ones-matmuls interleaved into the
  producing GEMM's PE stream; 1/sqrt(var+eps) via ACT Rsqrt; normalize
  is 3 bf16 DVE ops per feature tile, gating the next GEMM per-tile.
"""
import os
import numpy as np
from scipy.special import erf

V, H, NL, NH, S, B, HL, T = 30522, 768, 12, 12, 256, 32, 256, 9
DH = H // NH
FF = 4 * H
NCORES = 8
BL = B // NCORES          # sentences per core
TOK = BL * S              # tokens per core (1024)
KT = H // 128             # 6 k-tiles over hidden
MT_TOK = TOK // 128       # 8 token m-tiles
USE_DEVICE = os.environ.get("KERNEL_HOST", "") == ""
DEV_LAYERS = int(os.environ.get("KERNEL_LAYERS", str(NL)))

LAST_HW_NS = None
_CACHE = {}


# ---------------------------------------------------------------- host math
def _ln_np(x, g, b):
    m = x.mean(-1, keepdims=True)
    v = ((x - m) ** 2).mean(-1, keepdims=True)
    return (x - m) / np.sqrt(v + 1e-12) * g + b


def _gelu_np(x):
    return (0.5 * x * (1.0 + erf(x / np.float32(np.sqrt(2.0))))).astype(np.float32)


def _sigmoid_np(x):
    return 1.0 / (1.0 + np.exp(-x))


def _bert_host(h, a, n_layers=NL, l0=0):
    Bc = h.shape[0]
    for l in range(l0, l0 + n_layers):
        qkv = h @ a['Wqkv'][l] + a['bqkv'][l]
        q, k, v = [t.reshape(Bc, S, NH, DH) for t in np.split(qkv, 3, axis=-1)]
        sc = np.einsum('bqhd,bkhd->bhqk', q, k) / np.float32(np.sqrt(DH))
        sc = sc - sc.max(-1, keepdims=True)
        p = np.exp(sc)
        p = p / p.sum(-1, keepdims=True)
        ctx = np.einsum('bhqk,bkhd->bqhd', p, v).reshape(Bc, S, H)
        h = _ln_np(h + ctx @ a['Wo'][l] + a['bo'][l], a['ln1_g'][l], a['ln1_b'][l])
        ff = _gelu_np(h @ a['W1'][l] + a['b1'][l]) @ a['W2'][l] + a['b2'][l]
        h = _ln_np(h + ff, a['ln2_g'][l], a['ln2_b'][l])
    return h


def _lstm_host(x, Wih, Whh, bih, bhh, reverse):
    Bc = x.shape[0]
    pre = np.swapaxes(x, 0, 1) @ Wih.T + (bih + bhh)  # [S,B,4H]
    hs = np.zeros((S, Bc, HL), np.float32)
    h = np.zeros((Bc, HL), np.float32)
    c = np.zeros((Bc, HL), np.float32)
    order = range(S - 1, -1, -1) if reverse else range(S)
    for t in order:
        g = pre[t] + h @ Whh.T
        i, f, gg, o = np.split(g, 4, axis=-1)
        c = _sigmoid_np(f) * c + _sigmoid_np(i) * np.tanh(gg)
        h = _sigmoid_np(o) * np.tanh(c)
        hs[t] = h
    return np.swapaxes(hs, 0, 1)


def _logsumexp(a, axis):
    m = a.max(axis=axis, keepdims=True)
    return (np.log(np.exp(a - m).sum(axis=axis, keepdims=True)) + m).squeeze(axis)


def _crf_host(logits, labels, maskf, crf_start, crf_end, crf_trans):
    em = np.take_along_axis(logits, labels[..., None], -1)[..., 0]
    tr = crf_trans[labels[:, :-1], labels[:, 1:]]
    last_idx = maskf.sum(1).astype(np.int32) - 1
    last_tag = np.take_along_axis(labels, last_idx[:, None], 1)[:, 0]
    num = (crf_start[labels[:, 0]] + em[:, 0]
           + ((em[:, 1:] + tr) * maskf[:, 1:]).sum(1) + crf_end[last_tag])
    alpha = crf_start + logits[:, 0]
    for t in range(1, S):
        nxt = _logsumexp(alpha[:, :, None] + crf_trans[None] + logits[:, t][:, None, :], 1)
        alpha = np.where(maskf[:, t][:, None] > 0, nxt, alpha)
    den = _logsumexp(alpha + crf_end, -1)
    return den - num


# ------------------------------------------------------------ device program
class Prog:
    """Raw-Bass multi-engine program recorder with snapshot-based sync.

    Each op waits on an explicit {sem: value} dict built from count
    snapshots taken when its producers were emitted, so consumers wait
    only for what they actually need. Per-engine floors elide redundant
    waits. DMA uses one semaphore per weight slot / purpose so at most
    one transfer is outstanding per semaphore (exact count waits)."""

    def __init__(self):
        self.ops = {e: [] for e in ("pe", "act", "dve", "dma")}
        self.counts = {}              # sem name -> emitted count
        self.seen = {e: {} for e in self.ops}
        self.sem_names = {"pe", "act", "dve"}
        self.base = {}                # floor merged into every op's wait

    def snap(self):
        return dict(self.counts)

    def _sems_of(self, engine_key, source):
        if engine_key == "dma":
            return [s for s in source if s.startswith("dma")]
        return [engine_key] if engine_key in source else []

    def wait_of(self, snap_, *engine_keys):
        w = {}
        for k in engine_keys:
            for sem in self._sems_of(k, snap_):
                v = snap_[sem]
                if v > 0:
                    w[sem] = max(w.get(sem, 0), v)
        return w

    @staticmethod
    def merge(*waits):
        out = {}
        for w in waits:
            if not w:
                continue
            for sem, v in w.items():
                out[sem] = max(out.get(sem, 0), v)
        return out

    def emit(self, engine, fn, wait=None, deps=(), sem=None):
        w = dict(self.base)
        if wait:
            for s, v in wait.items():
                w[s] = max(w.get(s, 0), v)
        if deps:
            cur = self.counts
            for d in deps:
                for s in self._sems_of(d, cur):
                    w[s] = max(w.get(s, 0), cur[s])
        waits = []
        for s, val in w.items():
            if val > 0 and self.seen[engine].get(s, -1) < val:
                waits.append((s, val))
                self.seen[engine][s] = val
        sem_self = sem if sem else engine
        self.sem_names.add(sem_self)
        inc = 16 if engine == "dma" else 1
        self.counts[sem_self] = self.counts.get(sem_self, 0) + inc
        self.ops[engine].append((waits, fn, sem_self, inc))

    def replay(self, engine, eng, sems):
        for waits, fn, sem_self, inc in self.ops[engine]:
            for name, val in waits:
                eng.wait_ge(sems[name], val)
            fn().then_inc(sems[sem_self], inc)


def _build_encoder(n_layers):
    import concourse.bass as bass
    import concourse.mybir as mybir
    from contextlib import ExitStack
    dt = mybir.dt
    f32 = dt.float32
    bf16 = dt.bfloat16
    AF = mybir.ActivationFunctionType
    ALU = mybir.AluOpType
    GELU = (AF.Identity if os.environ.get("KERNEL_SIM_NOGELU", "")
            else AF.Gelu)   # CoreSim lacks Gelu; HW always uses the real one

    nc = bass.Bass()
    ctx = ExitStack()

    def R32(ap):
        return ap.bitcast(dt.float32r)

    # ---- DRAM parameters (weights pre-packed on host, bf16)
    hT0 = nc.declare_dram_parameter("hT0", [H, TOK], bf16, isOutput=False)
    Wqk = nc.declare_dram_parameter("Wqk", [NL, 12, 128, KT * 128], bf16, isOutput=False)
    Wv = nc.declare_dram_parameter("Wv", [NL, KT, 128, H], bf16, isOutput=False)
    Wo4 = nc.declare_dram_parameter("Wo4", [NL, KT, 128, KT * 128], bf16, isOutput=False)
    W14 = nc.declare_dram_parameter("W14", [NL, 24, 128, KT * 128], bf16, isOutput=False)
    W24 = nc.declare_dram_parameter("W24", [NL, KT, 128, 24 * 128], bf16, isOutput=False)
    biasall = nc.declare_dram_parameter("biasall", [NL, 128, 80], f32, isOutput=False)
    cbfd = nc.declare_dram_parameter("cbfd", [128, 1024], bf16, isOutput=False)
    cf32d = nc.declare_dram_parameter("cf32d", [128, 128], f32, isOutput=False)
    hTout = nc.declare_dram_parameter("hTout", [H, TOK], bf16, isOutput=True)
    zscr = nc.dram_tensor("zscr", [1, NH * TOK], bf16)

    # ---- on-chip tensors
    def sbt(nm, shape, d=bf16):
        return ctx.enter_context(nc.sbuf_tensor(nm, shape, d))

    hT = sbt("hT", [128, KT, TOK])
    h1T = sbt("h1T", [128, KT, TOK])
    ctxT = sbt("ctxT", [128, KT, TOK])
    big = sbt("bigb", [128, 12, TOK])       # q(0-5) k(6-11); FF1 0-11; LN2 sq
    ff1x = sbt("ff1x", [128, KT, TOK])      # LN1 squares; FF1 tiles 18-23
    vbuf = sbt("vbuf", [128, MT_TOK, NH, DH + 1])   # v + ones column
    wsl = sbt("wsl", [128, 16, KT * 128])   # weight slots (qkv/v/Wo/FF1)
    wff2 = sbt("wff2", [128, 4, 24 * 128])  # FF2 weight slots
    expS = sbt("expS", [128, 2, 4, S])      # exp(scores) pair slots; LN m/istd
    bias = sbt("biassb", [128, 80], f32)
    cbf = sbt("cbf", [128, 1024])
    cf32 = sbt("cf32", [128, 128], f32)
    sq2 = sbt("sq2", [128, KT, TOK])        # LN2 squares (FF2 still reads big)
    zflat = sbt("zflat", [1, NH * TOK])
    zbufT = sbt("zbufT", [NH, TOK])
    smean = sbt("smean", [1, TOK], f32)
    se2 = sbt("se2", [1, TOK], f32)
    sisd = sbt("sisd", [1, TOK], f32)

    psA = ctx.enter_context(nc.psum_tensor("psA", [128, 1024], f32))
    psB = ctx.enter_context(nc.psum_tensor("psB", [128, 1024], f32))
    psS = ctx.enter_context(nc.psum_tensor("psS", [128, 1024], f32))
    psT = ctx.enter_context(nc.psum_tensor("psT", [128, 1024], f32))

    P = Prog()
    W_, M_ = P.wait_of, P.merge

    ones128 = cbf[:, 0:1]
    onesrow32 = cf32[0:1, 0:128]

    def map12(kt):                    # [12, 128] head->feature map, k-tile kt
        return cbf[0:12, 130 + kt * 128:130 + (kt + 1) * 128]

    def dma(dst, src, qsem, wait=None, sparse_ok=False):
        def f(d=dst, s=src):
            if sparse_ok:
                with nc.allow_non_contiguous_dma(reason="tiny one-time fill"):
                    return nc.sync.dma_start(out=d, in_=s)
            return nc.sync.dma_start(out=d, in_=s)
        P.emit("dma", f, wait=wait, sem=qsem)

    def mm(out, lhsT, rhs, start, stop, wait=None, raw32=False):
        if raw32:
            lhsT, rhs = R32(lhsT), R32(rhs)
        P.emit("pe", lambda o=out, l=lhsT, r=rhs, a=start, b=stop:
               nc.tensor.matmul(o, l, r, start=a, stop=b), wait=wait)

    def act(out, in_, func, b=0.0, scale=1.0, wait=None, deps=("pe",)):
        P.emit("act", lambda o=out, i=in_, f=func, bb=b, s=scale:
               nc.scalar.activation(o, i, f, bias=bb, scale=s),
               wait=wait, deps=deps)

    def dve_tt(out, in0, in1, op, wait=None, deps=()):
        P.emit("dve", lambda o=out, x=in0, y=in1, z=op:
               nc.vector.tensor_tensor(o, x, y, z), wait=wait, deps=deps)

    def dve_ts(out, in_, s1, s2, op0, op1, wait=None, deps=()):
        P.emit("dve", lambda o=out, i=in_, a=s1, b=s2, x=op0, y=op1:
               nc.vector.tensor_scalar(o, i, a, b, x, y), wait=wait, deps=deps)

    def dve_tsadd(out, in_, s1, wait=None, deps=()):
        P.emit("dve", lambda o=out, i=in_, a=s1:
               nc.vector.tensor_scalar_add(o, i, a), wait=wait, deps=deps)

    def dve_copy(out, in_, wait=None, deps=()):
        P.emit("dve", lambda o=out, i=in_:
               nc.vector.tensor_copy(o, i), wait=wait, deps=deps)

    def dve_recip(out, in_, wait=None, deps=()):
        def f(o=out, i=in_):
            with nc.allow_low_precision(reason="bf16 z-recip; 2e-2 budget"):
                return nc.vector.reciprocal(o, i)
        P.emit("dve", f, wait=wait, deps=deps)

    # psum guards: writer waits for last reader's emission snapshot
    guard = {"psA": {}, "psB": {}, "psS": {}, "psT": {}}
    psum_pair = [("psA", psA), ("psB", psB)]

    # weight slot allocator over wsl's 16 sub-slots (per-slot DMA sems)
    slot_last_use = [{} for _ in range(16)]
    slot_next = [0]
    wff2_last = [None] * 4

    def slot_load(l_, src):
        s = slot_next[0]
        slot_next[0] = (s + 1) % 16
        last = slot_last_use[s]
        dma(wsl[:, s, :], src, f"dmaW{s}",
            wait=W_(last, "pe") if last else None)
        return s, P.snap()

    # ---- boot
    dma(cbf[:, :], cbfd[:, :], "dmaB")
    dma(R32(cf32[:, :]), R32(cf32d[:, :]), "dmaB")
    boot_c = P.snap()
    # v ones columns (persist across layers; FF1 does not touch vbuf)
    dma(vbuf[:, :, :, DH:DH + 1].rearrange("p a h o -> p (a h o)"),
        cbf[:, 1:1 + MT_TOK * NH], "dmaB", wait=W_(boot_c, "dma"),
        sparse_ok=True)
    dma(hT[:, :, :], hT0.rearrange("(n p) t -> p n t", p=128), "dmaB")
    boot = P.snap()
    gate_hT = [W_(boot, "dma") for _ in range(KT)]     # per-kt rhs gates
    bias_w = {}

    for l in range(n_layers):
        # layer floor: every op this layer waits at least for all work
        # emitted before the layer (free at runtime, satisfies the race
        # detector for cross-layer buffer reuse)
        P.base = P.wait_of(P.snap(), "pe", "act", "dve", "dma")
        dma(bias[:, :], biasall[l], "dmaB", wait=M_(bias_w))
        bw = W_(P.snap(), "dma")

        # ---------------- qkv (q,k) -> big[:, 0:12]
        qk_slots = [slot_load(l, Wqk[l, m]) for m in range(12)]
        for m in range(12):
            s, dsnap = qk_slots[m]
            pnm, ps = psum_pair[m % 2]
            w0 = M_(W_(dsnap, "dma"), W_(guard[pnm], "act"))
            for half in range(2):
                for kt in range(KT):
                    mm(ps[:, half * 512:(half + 1) * 512],
                       wsl[:, s, kt * 128:(kt + 1) * 128],
                       hT[:, kt, half * 512:(half + 1) * 512],
                       start=(kt == 0), stop=(kt == KT - 1),
                       wait=M_(w0, gate_hT[kt]))
            slot_last_use[s] = P.snap()
            act(big[:, m, :], ps[:, :], AF.Identity, b=bias[:, m:m + 1],
                wait=bw)
            guard[pnm] = P.snap()
        qk_done = P.snap()

        # ---------------- v (token-major, ones column already in place)
        v_slots = [slot_load(l, Wv[l, kt]) for kt in range(KT)]
        v_dma = M_(*[W_(d, "dma") for _, d in v_slots])
        for m in range(MT_TOK):
            pnm, ps = psum_pair[m % 2]
            w0 = M_(v_dma, W_(guard[pnm], "act"))
            for c0, c1 in ((0, 512), (512, 768)):
                for kt in range(KT):
                    mm(ps[:, c0:c1], hT[:, kt, m * 128:(m + 1) * 128],
                       wsl[:, v_slots[kt][0], c0:c1],
                       start=(kt == 0), stop=(kt == KT - 1),
                       wait=M_(w0, gate_hT[kt]))
            act(vbuf[:, m, :, 0:DH],
                ps[:, 0:H].rearrange("p (h d) -> p h d", d=DH),
                AF.Identity)
            guard[pnm] = P.snap()
        v_use = P.snap()
        for s, _ in v_slots:
            slot_last_use[s] = v_use

        # ---------------- attention: pairs (hp, s); z lands in psum row 64
        # scores pair p -> psA/psB alternating; ctx+z pair -> psS/psT with
        # column halves, so sentences (s, s+2) of one head pair share a psum
        # tensor and z drains in a single [1,1024] copy.
        zf4 = zflat[0:1, :].rearrange("p (h k r t) -> p h k r t",
                                      h=NH, k=2, r=2, t=S)
        ctx_tile_snaps = []
        for hp in range(6):
            for sidx in range(BL):
                p = hp * BL + sidx
                spn, sps = psum_pair[p % 2]
                cpn, cps = ("psS", psS) if p % 2 == 0 else ("psT", psT)
                chalf = (p % 4) // 2
                eslot = p % 2
                # scores: 2 heads at partition rows 0/64 run concurrently
                wsc = M_(W_(guard[spn], "act"), W_(qk_done, "act"))
                for j in range(2):
                    hh = 2 * hp + j
                    prow = 64 * (hh % 2)
                    qt = big[prow:prow + 64, hh // 2, sidx * S:(sidx + 1) * S]
                    ktap = big[prow:prow + 64, 6 + hh // 2,
                               sidx * S:(sidx + 1) * S]
                    for i in range(2):
                        mm(sps[:, (2 * j + i) * S:(2 * j + i + 1) * S],
                           ktap[:, i * 128:(i + 1) * 128], qt,
                           start=True, stop=True, wait=wsc)
                sc_snap = P.snap()
                act(expS[:, eslot, :, :],
                    sps[:, :].rearrange("p (a t) -> p a t", t=S),
                    AF.Exp, scale=1.0 / 8.0, wait=W_(sc_snap, "pe"))
                exp_snap = P.snap()
                guard[spn] = exp_snap
                # ctx (+z in row 64): accumulate over the 2 token tiles
                wctx = M_(W_(exp_snap, "act"), W_(guard[cpn], "dve", "act"))
                for j in range(2):
                    hh = 2 * hp + j
                    for i in range(2):
                        mm(cps[0:DH + 1,
                               chalf * 512 + j * S:chalf * 512 + (j + 1) * S],
                           vbuf[:, 2 * sidx + i, hh, 0:DH + 1],
                           expS[:, eslot, 2 * j + i, :],
                           start=(i == 0), stop=(i == 1), wait=wctx)
                ctx_snap = P.snap()
                # drain ctx rows 0:64 -> ctxT (per head, partition shift)
                for j in range(2):
                    dve_copy(ctxT[64 * j:64 * (j + 1), hp,
                                  sidx * S:(sidx + 1) * S],
                             cps[0:DH, chalf * 512 + j * S:
                                 chalf * 512 + (j + 1) * S],
                             wait=W_(ctx_snap, "pe", "act"))
                # drain z row 64 (both column halves) once both are filled
                if sidx >= 2:
                    r = p % 2
                    dve_copy(zf4[0:1, 2 * hp:2 * hp + 2, :, r:r + 1, :]
                             .rearrange("p a k o t -> p a k (o t)"),
                             cps[DH:DH + 1, :]
                             .rearrange("p (k j t) -> p j k t", k=2, j=2),
                             wait=W_(ctx_snap, "pe", "act"))
                guard[cpn] = P.snap()
            ctx_tile_snaps.append(P.snap())   # ctxT tile hp fully drained
        attn_done = P.snap()

        # ---------------- z transpose + reciprocal + normalize ctxT
        dma(zscr[0:1, :], zflat[0:1, :], "dmaZ", wait=W_(attn_done, "dve"))
        zst = P.snap()
        dma(zbufT[:, :], zscr[0:1, :].rearrange("o (h t) -> (o h) t", t=TOK),
            "dmaZ", wait=W_(zst, "dma"))
        zld = P.snap()
        dve_recip(zbufT[:, :], zbufT[:, :], wait=W_(zld, "dma"))
        zrec = P.snap()
        # pass 1: broadcast 1/z and multiply (per-tile dve floor = that
        # tile's attention drains, old by now -> no stall)
        zmul_snaps = []
        for kt in range(KT):
            pnm, ps = psum_pair[kt % 2]
            wz = M_(W_(zrec, "dve"), W_(guard[pnm], "dve", "act"))
            for half in range(2):
                mm(ps[:, half * 512:(half + 1) * 512], map12(kt),
                   zbufT[:, half * 512:(half + 1) * 512],
                   start=True, stop=True, wait=wz)
            zmm = P.snap()
            dve_tt(ctxT[:, kt, :], ctxT[:, kt, :], ps[:, :], ALU.mult,
                   wait=M_(W_(zmm, "pe"), W_(ctx_tile_snaps[kt], "dve")))
            guard[pnm] = P.snap()
            zmul_snaps.append(P.snap())
        # pass 2: add v bias (self-waits reference pass-1 counts, loose)
        gate_ctxT = []
        for kt in range(KT):
            dve_tsadd(ctxT[:, kt, :], ctxT[:, kt, :],
                      bias[:, 12 + kt:13 + kt],
                      wait=W_(zmul_snaps[kt], "dve"))
            gate_ctxT.append(W_(P.snap(), "dve"))

        # ---------------- Wo + residual; LN1 sums interleaved
        for m in range(KT):
            s, dsnap = slot_load(l, Wo4[l, m])
            pnm, ps = psum_pair[m % 2]
            w0 = M_(W_(dsnap, "dma"), W_(guard[pnm], "act"))
            for half in range(2):
                for kt in range(KT):
                    mm(ps[:, half * 512:(half + 1) * 512],
                       wsl[:, s, kt * 128:(kt + 1) * 128],
                       ctxT[:, kt, half * 512:(half + 1) * 512],
                       start=(kt == 0), stop=(kt == KT - 1),
                       wait=M_(w0, gate_ctxT[kt]))
            slot_last_use[s] = P.snap()
            act(h1T[:, m, :], ps[:, :], AF.Identity, b=bias[:, 18 + m:19 + m],
                wait=bw)
            guard[pnm] = P.snap()
            dve_tt(h1T[:, m, :], h1T[:, m, :], hT[:, m, :], ALU.add,
                   deps=("act",))
            res_snap = P.snap()
            act(ff1x[:, m, :], h1T[:, m, :], AF.Square,
                wait=W_(res_snap, "dve"))
            sq_snap = P.snap()
            wsum = M_(W_(res_snap, "dve"),
                      W_(guard["psS"], "act", "dve") if m == 0 else {})
            wsq = M_(W_(sq_snap, "act"),
                     W_(guard["psT"], "act", "dve") if m == 0 else {})
            for half in range(2):
                mm(psS[0:1, half * 512:(half + 1) * 512], ones128,
                   h1T[:, m, half * 512:(half + 1) * 512],
                   start=(m == 0), stop=(m == KT - 1), wait=wsum)
                mm(psT[0:1, half * 512:(half + 1) * 512], ones128,
                   ff1x[:, m, half * 512:(half + 1) * 512],
                   start=(m == 0), stop=(m == KT - 1), wait=wsq)
        sums1 = P.snap()

        # ---------------- LN scalar chain + normalize (shared LN1/LN2)
        def layernorm(x, gcol, bcol, sums_snap):
            act(R32(smean[0:1, :]), psS[0:1, :], AF.Identity, scale=1.0 / H,
                wait=W_(sums_snap, "pe"))
            act(se2[0:1, :], psT[0:1, :], AF.Identity, scale=1.0 / H)
            st1 = P.snap()
            dve_tt(R32(sisd[0:1, :]), smean[0:1, :], smean[0:1, :], ALU.mult,
                   wait=M_(W_(st1, "act"), W_(sums_snap, "dve")))
            dve_tt(se2[0:1, :], se2[0:1, :], sisd[0:1, :], ALU.subtract,
                   deps=("dve",))
            P.emit("dve", lambda: nc.vector.tensor_scalar_add(
                se2[0:1, :], se2[0:1, :], 1e-12), deps=("dve",))
            st2 = P.snap()
            act(R32(sisd[0:1, :]), se2[0:1, :], AF.Sqrt,
                wait=W_(st2, "dve"))
            stq = P.snap()
            dve_recip(R32(sisd[0:1, :]), sisd[0:1, :], wait=W_(stq, "act"))
            st3 = P.snap()
            guard["psS"] = st3
            guard["psT"] = st3
            wb = M_(W_(st3, "act", "dve"), W_(guard["psA"], "act", "dve"),
                    W_(guard["psB"], "act", "dve"))
            for half in range(2):
                c0, c1 = half * 512, (half + 1) * 512
                mm(psA[:, c0:c1], onesrow32, smean[0:1, c0:c1],
                   start=True, stop=True, wait=wb, raw32=True)
                mm(psB[:, c0:c1], onesrow32, sisd[0:1, c0:c1],
                   start=True, stop=True, wait=wb, raw32=True)
            bc = P.snap()
            mbuf = expS[:, 0, :, :].rearrange("p a t -> p (a t)")
            ibuf = expS[:, 1, :, :].rearrange("p a t -> p (a t)")
            act(mbuf, psA[:, :], AF.Identity, wait=W_(bc, "pe"))
            act(ibuf, psB[:, :], AF.Identity)
            cp = P.snap()
            guard["psA"] = cp
            guard["psB"] = cp
            # three passes so same-tile self-waits reference loose counts
            sub_snaps, mul_snaps, gates = [], [], []
            for kt in range(KT):
                dve_tt(x[:, kt, :], x[:, kt, :], mbuf, ALU.subtract,
                       wait=M_(W_(cp, "act"), W_(sums_snap, "dve")))
                sub_snaps.append(P.snap())
            for kt in range(KT):
                dve_tt(x[:, kt, :], x[:, kt, :], ibuf, ALU.mult,
                       wait=W_(sub_snaps[kt], "dve"))
                mul_snaps.append(P.snap())
            for kt in range(KT):
                dve_ts(x[:, kt, :], x[:, kt, :],
                       bias[:, gcol + kt:gcol + kt + 1],
                       bias[:, bcol + kt:bcol + kt + 1], ALU.mult, ALU.add,
                       wait=W_(mul_snaps[kt], "dve"))
                gates.append(W_(P.snap(), "dve"))
            return gates

        gate_h1T = layernorm(h1T, 24, 30, sums1)

        # ---------------- FF1 (gelu) -> big[0:12] + ctxT + ff1x
        def fftile(m):
            if m < 12:
                return big[:, m, :]
            if m < 18:
                return ctxT[:, m - 12, :]
            return ff1x[:, m - 18, :]

        ff1_gate = []
        for m in range(24):
            s, dsnap = slot_load(l, W14[l, m])
            pnm, ps = psum_pair[m % 2]
            w0 = M_(W_(dsnap, "dma"), W_(guard[pnm], "act"))
            for half in range(2):
                for kt in range(KT):
                    mm(ps[:, half * 512:(half + 1) * 512],
                       wsl[:, s, kt * 128:(kt + 1) * 128],
                       h1T[:, kt, half * 512:(half + 1) * 512],
                       start=(kt == 0), stop=(kt == KT - 1),
                       wait=M_(w0, gate_h1T[kt]))
            slot_last_use[s] = P.snap()
            act(fftile(m), ps[:, :], GELU, b=bias[:, 36 + m:37 + m],
                wait=bw)
            guard[pnm] = P.snap()
            ff1_gate.append(W_(P.snap(), "act"))

        # ---------------- FF2 + residual -> hT; LN2 sums interleaved
        ff2_dma = [None] * KT
        for m in range(4):
            last = wff2_last[m]
            dma(wff2[:, m, :], W24[l, m], f"dmaF{m}",
                wait=W_(last, "pe") if last else None)
            ff2_dma[m] = P.snap()
        for m in range(KT):
            si = m % 4
            pnm, ps = psum_pair[m % 2]
            w0 = M_(W_(ff2_dma[m], "dma"), W_(guard[pnm], "act"))
            for half in range(2):
                for kt in range(24):
                    mm(ps[:, half * 512:(half + 1) * 512],
                       wff2[:, si, kt * 128:(kt + 1) * 128],
                       fftile(kt)[:, half * 512:(half + 1) * 512],
                       start=(kt == 0), stop=(kt == 23),
                       wait=M_(w0, ff1_gate[kt]))
            wff2_last[si] = P.snap()
            if m + 4 < KT:      # prefetch the slot-reusing tile's weights
                nm = m + 4
                dma(wff2[:, nm % 4, :], W24[l, nm], f"dmaF{nm % 4}",
                    wait=W_(wff2_last[nm % 4], "pe"))
                ff2_dma[nm] = P.snap()
            act(hT[:, m, :], ps[:, :], AF.Identity, b=bias[:, 60 + m:61 + m],
                wait=bw)
            guard[pnm] = P.snap()
            dve_tt(hT[:, m, :], hT[:, m, :], h1T[:, m, :], ALU.add,
                   wait=gate_h1T[m], deps=("act",))
            res_snap = P.snap()
            act(sq2[:, m, :], hT[:, m, :], AF.Square,
                wait=W_(res_snap, "dve"))
            sq_snap = P.snap()
            wsum = M_(W_(res_snap, "dve"),
                      W_(guard["psS"], "act", "dve") if m == 0 else {})
            wsq = M_(W_(sq_snap, "act"),
                     W_(guard["psT"], "act", "dve") if m == 0 else {})
            for half in range(2):
                mm(psS[0:1, half * 512:(half + 1) * 512], ones128,
                   hT[:, m, half * 512:(half + 1) * 512],
                   start=(m == 0), stop=(m == KT - 1), wait=wsum)
                mm(psT[0:1, half * 512:(half + 1) * 512], ones128,
                   sq2[:, m, half * 512:(half + 1) * 512],
                   start=(m == 0), stop=(m == KT - 1), wait=wsq)
        sums2 = P.snap()

        gate_hT = layernorm(hT, 66, 72, sums2)
        bias_w = W_(P.snap(), "act", "dve")

    fin = M_(*gate_hT)
    dma(hTout.rearrange("(n p) t -> p n t", p=128), hT[:, :, :], "dmaB",
        wait=fin)

    # ---- replay into engine blocks
    sems = {}
    for name in sorted(P.sem_names):
        sems[name] = ctx.enter_context(nc.semaphore(name))
    with nc.Block() as block:
        @block.tensor
        def _(eng):
            P.replay("pe", eng, sems)

        @block.scalar
        def _(eng):
            P.replay("act", eng, sems)

        @block.vector
        def _(eng):
            P.replay("dve", eng, sems)

        @block.sync
        def _(eng):
            P.replay("dma", eng, sems)

    return nc, ctx


def _pack_consts_bf():
    import ml_dtypes
    c = np.zeros((128, 1024), np.float32)
    c[:, 0] = 1.0                         # ones128
    c[:, 1:1 + MT_TOK * NH] = 1.0         # v ones-column fill source
    for kt in range(KT):
        for f in range(128):
            hh = (kt * 128 + f) // DH
            c[hh, 130 + kt * 128 + f] = 1.0
    return c.astype(ml_dtypes.bfloat16)


def _pack_consts_f32():
    c = np.zeros((128, 128), np.float32)
    c[0, :] = 1.0                         # onesrow32
    return c


def _pack_bias(a):
    out = np.zeros((NL, 128, 80), np.float32)

    def col(vec):                       # feature vec [n*128] -> [128, n]
        return vec.reshape(-1, 128).T

    for l in range(NL):
        out[l, :, 0:18] = col(a['bqkv'][l])
        out[l, :, 18:24] = col(a['bo'][l])
        out[l, :, 24:30] = col(a['ln1_g'][l])
        out[l, :, 30:36] = col(a['ln1_b'][l])
        out[l, :, 36:60] = col(a['b1'][l])
        out[l, :, 60:66] = col(a['b2'][l])
        out[l, :, 66:72] = col(a['ln2_g'][l])
        out[l, :, 72:78] = col(a['ln2_b'][l])
    return out


def _pack_weights(a):
    """Pre-pack weights into m-tile-contiguous bf16 slabs:
    slab[l, m, r, kt*128+c] = W[l, kt*128+r, m*128+c]."""
    import ml_dtypes
    bf = ml_dtypes.bfloat16

    def slab(w, n_in, n_out):
        ktn, mtn = n_in // 128, n_out // 128
        return np.ascontiguousarray(
            np.asarray(w).reshape(NL, ktn, 128, mtn, 128)
            .transpose(0, 3, 2, 1, 4).reshape(NL, mtn, 128, ktn * 128)
        ).astype(bf)

    Wqk = slab(np.ascontiguousarray(a['Wqkv'][:, :, :12 * 128]), H, 12 * 128)
    Wv = np.ascontiguousarray(
        a['Wqkv'][:, :, 12 * 128:18 * 128].reshape(NL, KT, 128, H)).astype(bf)
    Wo4 = slab(a['Wo'], H, H)
    W14 = slab(a['W1'], H, FF)
    W24 = slab(a['W2'], FF, H)
    return {"Wqk": Wqk, "Wv": Wv, "Wo4": Wo4, "W14": W14, "W24": W24}


def _profile_ntff(nc, run_fn):
    """Re-run `run_fn` under NRT/NTFF profiling (core 0) and return
    (results, exec_time_ns, trace_path); (results, None, None) if the
    profiling stack is unavailable. neuron-profile measures only the NEFF
    execution on the device, so the returned time is pure HW exec time."""
    import ctypes
    import tempfile

    try:
        lib = ctypes.CDLL("/opt/axon/libaxon_pjrt.so")
        if not hasattr(lib, "axon_start_nrt_profile"):
            return run_fn(), None, None
    except OSError:
        return run_fn(), None, None
    lib.axon_start_nrt_profile.argtypes = [ctypes.POINTER(ctypes.c_int64),
                                           ctypes.c_size_t]
    lib.axon_start_nrt_profile.restype = ctypes.c_int64
    lib.axon_stop_nrt_profile.argtypes = [ctypes.c_char_p]
    lib.axon_stop_nrt_profile.restype = ctypes.c_int64

    import jax
    jax.devices()
    neff_dir = tempfile.mkdtemp(prefix="bassprof_")
    ids = (ctypes.c_int64 * 1)(0)
    if lib.axon_start_nrt_profile(ids, 1) != 0:
        return run_fn(), None, None
    try:
        results = run_fn()
    finally:
        nfiles = lib.axon_stop_nrt_profile(neff_dir.encode())
    if nfiles <= 0:
        return results, None, None
    try:
        from concourse._compat import FishPath
        import gauge.profiler
        profile = gauge.profiler.Profile(
            profile_path=FishPath(neff_dir),
            kernel_dev_mode=True,
            profile_on_exit=False,
            bass_kernel=nc.m,
            offline_processing=True,
            fname="*_body*",
        )
        pres = profile.to_perfetto(model_index=(0,))
        if pres and pres[0].exec_time_ns:
            return results, int(pres[0].exec_time_ns), pres[0].trace_path
    except Exception as e:
        print("[kernel] ntff processing failed:", e)
    return results, None, None


def run_device(h0, a):
    global LAST_HW_NS
    if not USE_DEVICE:
        return _bert_host(h0, a)
    import time
    import ml_dtypes
    from concourse.bass_utils import run_bass_kernel_spmd

    key = ("enc", DEV_LAYERS)
    if key not in _CACHE:
        _CACHE[key] = _build_encoder(DEV_LAYERS)
    nc, _ctx = _CACHE[key]

    shared = dict(_pack_weights(a))
    shared["biasall"] = _pack_bias(a)
    shared["cbfd"] = _pack_consts_bf()
    shared["cf32d"] = _pack_consts_f32()
    in_maps = []
    for c in range(NCORES):
        hc = h0[c * BL:(c + 1) * BL].reshape(TOK, H).T  # [H, TOK]
        in_maps.append(dict(shared, hT0=np.ascontiguousarray(hc)
                            .astype(ml_dtypes.bfloat16)))

    cores = list(range(NCORES))
    t0 = time.time()
    res = run_bass_kernel_spmd(nc, in_maps, cores)  # compile + warm run
    warm_wall_ns = int((time.time() - t0) * 1e9)
    LAST_HW_NS = warm_wall_ns
    if getattr(res, "exec_time_ns", None):
        LAST_HW_NS = int(res.exec_time_ns)

    if os.environ.get("KERNEL_NO_PROFILE", "") == "":
        try:
            res2, exec_ns, trace = _profile_ntff(
                nc, lambda: run_bass_kernel_spmd(nc, in_maps, cores))
            if exec_ns:
                res = res2
                LAST_HW_NS = exec_ns
                print("[kernel] profile exec_time_ns:", exec_ns,
                      "trace:", trace)
        except Exception as e:
            print("[kernel] profiling failed, using wall time:", e)

    h = np.zeros((B, S, H), np.float32)
    for c in range(NCORES):
        h[c * BL:(c + 1) * BL] = (res.results[c]["hTout"].astype(np.float32)
                                  .T.reshape(BL, S, H))
    if DEV_LAYERS < NL:                 # debugging path: finish on host
        h = _bert_host(h, a, NL - DEV_LAYERS, l0=DEV_LAYERS)
    return h


def kernel(input_ids, attention_mask, labels, emb_tok, emb_pos, emb_type,
           ln_emb_g, ln_emb_b, Wqkv, bqkv, Wo, bo, ln1_g, ln1_b, W1, b1,
           W2, b2, ln2_g, ln2_b, Wih_f, Whh_f, bih_f, bhh_f, Wih_b, Whh_b,
           bih_b, bhh_b, Wc, bc, tag_weight, crf_start, crf_end, crf_trans):
    args = {k: np.asarray(v) for k, v in locals().items()}
    maskf = args['attention_mask'].astype(np.float32)

    h0 = (args['emb_tok'][args['input_ids']] + args['emb_pos'][:S][None]
          + args['emb_type'][0][None, None]).astype(np.float32)
    h0 = _ln_np(h0, args['ln_emb_g'], args['ln_emb_b'])

    h = run_device(h0, args)

    hf = _lstm_host(h, args['Wih_f'], args['Whh_f'], args['bih_f'], args['bhh_f'], False)
    hb = _lstm_host(h, args['Wih_b'], args['Whh_b'], args['bih_b'], args['bhh_b'], True)
    logits = (np.concatenate([hf, hb], -1) @ args['Wc'] + args['bc']) * args['tag_weight']
    ll = _crf_host(logits, args['labels'], maskf, args['crf_start'],
                   args['crf_end'], args['crf_trans'])
    return np.float32(ll.mean())


# revision 46
# speedup vs baseline: 2.6982x; 1.0551x over previous
"""BertBiLSTMCRF loss kernel for 8 Trainium2 NeuronCores.

Sharding: data-parallel over batch (B=32 -> 4 sentences/core). The BERT
encoder (>95% of FLOPs) runs on-device in raw Bass; embeddings, the
BiLSTM and the CRF (small FLOPs, serial scans) run on host.

Device kernel design (v2):
- Activations and weights are bf16 on chip (fp32 PSUM accumulate, fp32
  LN statistics). Halves DMA traffic and doubles DVE throughput; PE rate
  matches fp32r while numerics stay far inside the 2e-2 budget.
- Activations live transposed, hT = [feature, token], so every GEMM
  consumes weights in stored [in,out] layout as lhsT with no activation
  transposes.
- Weights are host-packed so each m-tile's slab is ONE contiguous DMA,
  streamed through rotating SBUF slots with deep prefetch. Every weight
  slot has its own DMA semaphore so at most one transfer is ever
  outstanding per semaphore -- count waits are then exact even though
  SDMA completions reorder across engines.
- Fine-grained cross-engine sync: ops wait only on snapshot counts of
  the semaphores they actually depend on, with PSUM double-buffering
  (psA/psB) so the PE streams matmuls continuously while ACT/DVE drain
  behind it and DMA prefetches ahead. This also keeps the PE HAM
  clock-gate warm (2.4 GHz) instead of the 1.2 GHz it falls to when the
  PE idles between bursts (the baseline spent 75% of its time there).
- Attention: the two heads of a pair compute scores via concurrent
  row-group matmuls (partition bases 0/64), one exp covers the whole
  pair, and the softmax denominator comes free from a ones column
  appended to v (it lands in row 64 of the ctx psum). Unnormalized exp
  is safe here (LN'd inputs, 0.02-scale weights).
- LayerNorm: mean/sq sums are ones-matmuls interleaved into the
  producing GEMM's PE stream; 1/sqrt(var+eps) via ACT Rsqrt; normalize
  is 3 bf16 DVE ops per feature tile, gating the next GEMM per-tile.
"""
import os
import numpy as np
from scipy.special import erf

V, H, NL, NH, S, B, HL, T = 30522, 768, 12, 12, 256, 32, 256, 9
DH = H // NH
FF = 4 * H
NCORES = 8
BL = B // NCORES          # sentences per core
TOK = BL * S              # tokens per core (1024)
KT = H // 128             # 6 k-tiles over hidden
MT_TOK = TOK // 128       # 8 token m-tiles
USE_DEVICE = os.environ.get("KERNEL_HOST", "") == ""
DEV_LAYERS = int(os.environ.get("KERNEL_LAYERS", str(NL)))

LAST_HW_NS = None
_CACHE = {}


# ---------------------------------------------------------------- host math
def _ln_np(x, g, b):
    m = x.mean(-1, keepdims=True)
    v = ((x - m) ** 2).mean(-1, keepdims=True)
    return (x - m) / np.sqrt(v + 1e-12) * g + b


def _gelu_np(x):
    return (0.5 * x * (1.0 + erf(x / np.float32(np.sqrt(2.0))))).astype(np.float32)


def _sigmoid_np(x):
    return 1.0 / (1.0 + np.exp(-x))


def _bert_host(h, a, n_layers=NL, l0=0):
    Bc = h.shape[0]
    for l in range(l0, l0 + n_layers):
        qkv = h @ a['Wqkv'][l] + a['bqkv'][l]
        q, k, v = [t.reshape(Bc, S, NH, DH) for t in np.split(qkv, 3, axis=-1)]
        sc = np.einsum('bqhd,bkhd->bhqk', q, k) / np.float32(np.sqrt(DH))
        sc = sc - sc.max(-1, keepdims=True)
        p = np.exp(sc)
        p = p / p.sum(-1, keepdims=True)
        ctx = np.einsum('bhqk,bkhd->bqhd', p, v).reshape(Bc, S, H)
        h = _ln_np(h + ctx @ a['Wo'][l] + a['bo'][l], a['ln1_g'][l], a['ln1_b'][l])
        ff = _gelu_np(h @ a['W1'][l] + a['b1'][l]) @ a['W2'][l] + a['b2'][l]
        h = _ln_np(h + ff, a['ln2_g'][l], a['ln2_b'][l])
    return h


def _lstm_host(x, Wih, Whh, bih, bhh, reverse):
    Bc = x.shape[0]
    pre = np.swapaxes(x, 0, 1) @ Wih.T + (bih + bhh)  # [S,B,4H]
    hs = np.zeros((S, Bc, HL), np.float32)
    h = np.zeros((Bc, HL), np.float32)
    c = np.zeros((Bc, HL), np.float32)
    order = range(S - 1, -1, -1) if reverse else range(S)
    for t in order:
        g = pre[t] + h @ Whh.T
        i, f, gg, o = np.split(g, 4, axis=-1)
        c = _sigmoid_np(f) * c + _sigmoid_np(i) * np.tanh(gg)
        h = _sigmoid_np(o) * np.tanh(c)
        hs[t] = h
    return np.swapaxes(hs, 0, 1)


def _logsumexp(a, axis):
    m = a.max(axis=axis, keepdims=True)
    return (np.log(np.exp(a - m).sum(axis=axis, keepdims=True)) + m).squeeze(axis)


def _crf_host(logits, labels, maskf, crf_start, crf_end, crf_trans):
    em = np.take_along_axis(logits, labels[..., None], -1)[..., 0]
    tr = crf_trans[labels[:, :-1], labels[:, 1:]]
    last_idx = maskf.sum(1).astype(np.int32) - 1
    last_tag = np.take_along_axis(labels, last_idx[:, None], 1)[:, 0]
    num = (crf_start[labels[:, 0]] + em[:, 0]
           + ((em[:, 1:] + tr) * maskf[:, 1:]).sum(1) + crf_end[last_tag])
    alpha = crf_start + logits[:, 0]
    for t in range(1, S):
        nxt = _logsumexp(alpha[:, :, None] + crf_trans[None] + logits[:, t][:, None, :], 1)
        alpha = np.where(maskf[:, t][:, None] > 0, nxt, alpha)
    den = _logsumexp(alpha + crf_end, -1)
    return den - num


# ------------------------------------------------------------ device program
class Prog:
    """Raw-Bass multi-engine program recorder with snapshot-based sync.

    Each op waits on an explicit {sem: value} dict built from count
    snapshots taken when its producers were emitted, so consumers wait
    only for what they actually need. Per-engine floors elide redundant
    waits. DMA uses one semaphore per weight slot / purpose so at most
    one transfer is outstanding per semaphore (exact count waits)."""

    def __init__(self):
        self.ops = {e: [] for e in ("pe", "act", "dve", "dma")}
        self.counts = {}              # sem name -> emitted count
        self.seen = {e: {} for e in self.ops}
        self.sem_names = {"pe", "act", "dve"}
        self.base = {}                # floor merged into every op's wait

    def snap(self):
        return dict(self.counts)

    def _sems_of(self, engine_key, source):
        if engine_key == "dma":
            return [s for s in source if s.startswith("dma")]
        return [engine_key] if engine_key in source else []

    def wait_of(self, snap_, *engine_keys):
        w = {}
        for k in engine_keys:
            for sem in self._sems_of(k, snap_):
                v = snap_[sem]
                if v > 0:
                    w[sem] = max(w.get(sem, 0), v)
        return w

    @staticmethod
    def merge(*waits):
        out = {}
        for w in waits:
            if not w:
                continue
            for sem, v in w.items():
                out[sem] = max(out.get(sem, 0), v)
        return out

    def emit(self, engine, fn, wait=None, deps=(), sem=None):
        w = dict(self.base)
        if wait:
            for s, v in wait.items():
                w[s] = max(w.get(s, 0), v)
        if deps:
            cur = self.counts
            for d in deps:
                for s in self._sems_of(d, cur):
                    w[s] = max(w.get(s, 0), cur[s])
        waits = []
        for s, val in w.items():
            if val > 0 and self.seen[engine].get(s, -1) < val:
                waits.append((s, val))
                self.seen[engine][s] = val
        sem_self = sem if sem else engine
        self.sem_names.add(sem_self)
        inc = 16 if engine == "dma" else 1
        self.counts[sem_self] = self.counts.get(sem_self, 0) + inc
        self.ops[engine].append((waits, fn, sem_self, inc))

    def replay(self, engine, eng, sems):
        for waits, fn, sem_self, inc in self.ops[engine]:
            for name, val in waits:
                eng.wait_ge(sems[name], val)
            fn().then_inc(sems[sem_self], inc)


def _build_encoder(n_layers):
    import concourse.bass as bass
    import concourse.mybir as mybir
    from contextlib import ExitStack
    dt = mybir.dt
    f32 = dt.float32
    bf16 = dt.bfloat16
    AF = mybir.ActivationFunctionType
    ALU = mybir.AluOpType
    GELU = (AF.Identity if os.environ.get("KERNEL_SIM_NOGELU", "")
            else AF.Gelu)   # CoreSim lacks Gelu; HW always uses the real one

    nc = bass.Bass()
    ctx = ExitStack()

    def R32(ap):
        return ap.bitcast(dt.float32r)

    # ---- DRAM parameters (weights pre-packed on host, bf16)
    hT0 = nc.declare_dram_parameter("hT0", [H, TOK], bf16, isOutput=False)
    Wqk = nc.declare_dram_parameter("Wqk", [NL, 12, 128, KT * 128], bf16, isOutput=False)
    Wv = nc.declare_dram_parameter("Wv", [NL, KT, 128, H], bf16, isOutput=False)
    Wo4 = nc.declare_dram_parameter("Wo4", [NL, KT, 128, KT * 128], bf16, isOutput=False)
    W14 = nc.declare_dram_parameter("W14", [NL, 24, 128, KT * 128], bf16, isOutput=False)
    W24 = nc.declare_dram_parameter("W24", [NL, KT, 128, 24 * 128], bf16, isOutput=False)
    biasall = nc.declare_dram_parameter("biasall", [NL, 128, 80], f32, isOutput=False)
    cbfd = nc.declare_dram_parameter("cbfd", [128, 1024], bf16, isOutput=False)
    cf32d = nc.declare_dram_parameter("cf32d", [128, 128], f32, isOutput=False)
    hTout = nc.declare_dram_parameter("hTout", [H, TOK], bf16, isOutput=True)
    zscr = nc.dram_tensor("zscr", [1, NH * TOK], bf16)

    # ---- on-chip tensors
    def sbt(nm, shape, d=bf16):
        return ctx.enter_context(nc.sbuf_tensor(nm, shape, d))

    hT = sbt("hT", [128, KT, TOK])
    h1T = sbt("h1T", [128, KT, TOK])
    ctxT = sbt("ctxT", [128, KT, TOK])
    big = sbt("bigb", [128, 12, TOK])       # q(0-5) k(6-11); FF1 0-11; LN2 sq
    ff1x = sbt("ff1x", [128, KT, TOK])      # LN1 squares; FF1 tiles 18-23
    vbuf = sbt("vbuf", [128, MT_TOK, NH, DH + 1])   # v + ones column
    wsl = sbt("wsl", [128, 16, KT * 128])   # weight slots (qkv/v/Wo/FF1)
    wff2 = sbt("wff2", [128, 4, 24 * 128])  # FF2 weight slots
    expS = sbt("expS", [128, 2, 4, S])      # exp(scores) pair slots; LN m/istd
    bias = sbt("biassb", [128, 80], f32)
    cbf = sbt("cbf", [128, 1024])
    cf32 = sbt("cf32", [128, 128], f32)
    sq2 = sbt("sq2", [128, KT, TOK])        # LN2 squares (FF2 still reads big)
    zflat = sbt("zflat", [1, NH * TOK])
    zbufT = sbt("zbufT", [NH, TOK])
    zlnf = sbt("zlnf", [NH, TOK], f32)
    smean = sbt("smean", [1, TOK], f32)
    se2 = sbt("se2", [1, TOK], f32)
    sisd = sbt("sisd", [1, TOK], f32)

    psA = ctx.enter_context(nc.psum_tensor("psA", [128, 1024], f32))
    psB = ctx.enter_context(nc.psum_tensor("psB", [128, 1024], f32))
    psS = ctx.enter_context(nc.psum_tensor("psS", [128, 1024], f32))
    psT = ctx.enter_context(nc.psum_tensor("psT", [128, 1024], f32))

    P = Prog()
    W_, M_ = P.wait_of, P.merge

    ones128 = cbf[:, 0:1]
    onesrow32 = cf32[0:1, 0:128]

    def map12(kt):                    # [12, 128] head->feature map, k-tile kt
        return cbf[0:12, 130 + kt * 128:130 + (kt + 1) * 128]

    def dma(dst, src, qsem, wait=None, sparse_ok=False):
        def f(d=dst, s=src):
            if sparse_ok:
                with nc.allow_non_contiguous_dma(reason="tiny one-time fill"):
                    return nc.sync.dma_start(out=d, in_=s)
            return nc.sync.dma_start(out=d, in_=s)
        P.emit("dma", f, wait=wait, sem=qsem)

    def mm(out, lhsT, rhs, start, stop, wait=None, raw32=False):
        if raw32:
            lhsT, rhs = R32(lhsT), R32(rhs)
        P.emit("pe", lambda o=out, l=lhsT, r=rhs, a=start, b=stop:
               nc.tensor.matmul(o, l, r, start=a, stop=b), wait=wait)

    def act(out, in_, func, b=0.0, scale=1.0, wait=None, deps=("pe",)):
        P.emit("act", lambda o=out, i=in_, f=func, bb=b, s=scale:
               nc.scalar.activation(o, i, f, bias=bb, scale=s),
               wait=wait, deps=deps)

    def dve_tt(out, in0, in1, op, wait=None, deps=()):
        P.emit("dve", lambda o=out, x=in0, y=in1, z=op:
               nc.vector.tensor_tensor(o, x, y, z), wait=wait, deps=deps)

    def dve_ts(out, in_, s1, s2, op0, op1, wait=None, deps=()):
        P.emit("dve", lambda o=out, i=in_, a=s1, b=s2, x=op0, y=op1:
               nc.vector.tensor_scalar(o, i, a, b, x, y), wait=wait, deps=deps)

    def dve_tsadd(out, in_, s1, wait=None, deps=()):
        P.emit("dve", lambda o=out, i=in_, a=s1:
               nc.vector.tensor_scalar_add(o, i, a), wait=wait, deps=deps)

    def dve_copy(out, in_, wait=None, deps=()):
        P.emit("dve", lambda o=out, i=in_:
               nc.vector.tensor_copy(o, i), wait=wait, deps=deps)

    def dve_recip(out, in_, wait=None, deps=()):
        def f(o=out, i=in_):
            with nc.allow_low_precision(reason="bf16 z-recip; 2e-2 budget"):
                return nc.vector.reciprocal(o, i)
        P.emit("dve", f, wait=wait, deps=deps)

    # psum guards: writer waits for last reader's emission snapshot
    guard = {"psA": {}, "psB": {}, "psS": {}, "psT": {}}
    psum_pair = [("psA", psA), ("psB", psB)]

    def warm_mm(wait):
        # tiny matmul into a scratch psum row, gated on a serial-chain step:
        # keeps the PE HAM activity window busy so matmuls stay at 2.4 GHz
        # across ACT/DVE-only stretches (>3.4us of PE idle re-throttles)
        mm(psA[0:1, 0:64], ones128, cbf[:, 0:64], start=True, stop=True,
           wait=M_(wait, W_(guard["psA"], "act", "dve")))

    # weight slot allocator over wsl's 16 sub-slots (per-slot DMA sems)
    slot_last_use = [{} for _ in range(16)]
    slot_next = [0]
    wff2_last = [None] * 4

    def slot_load(l_, src):
        s = slot_next[0]
        slot_next[0] = (s + 1) % 16
        last = slot_last_use[s]
        dma(wsl[:, s, :], src, f"dmaW{s}",
            wait=W_(last, "pe") if last else None)
        return s, P.snap()

    # ---- boot
    dma(cbf[:, :], cbfd[:, :], "dmaB")
    dma(R32(cf32[:, :]), R32(cf32d[:, :]), "dmaB")
    boot_c = P.snap()
    # v ones columns (persist across layers; FF1 does not touch vbuf)
    dma(vbuf[:, :, :, DH:DH + 1].rearrange("p a h o -> p (a h o)"),
        cbf[:, 1:1 + MT_TOK * NH], "dmaB", wait=W_(boot_c, "dma"),
        sparse_ok=True)
    dma(hT[:, :, :], hT0.rearrange("(n p) t -> p n t", p=128), "dmaB")
    boot = P.snap()
    gate_hT = [W_(boot, "dma") for _ in range(KT)]     # per-kt rhs gates
    bias_w = {}

    for l in range(n_layers):
        # layer floor: every op this layer waits at least for all work
        # emitted before the layer (free at runtime, satisfies the race
        # detector for cross-layer buffer reuse)
        P.base = P.wait_of(P.snap(), "pe", "act", "dve", "dma")
        dma(bias[:, :], biasall[l], "dmaB", wait=M_(bias_w))
        bw = W_(P.snap(), "dma")

        # ---------------- qkv (q,k) -> big[:, 0:12]
        qk_slots = [slot_load(l, Wqk[l, m]) for m in range(12)]
        for m in range(12):
            s, dsnap = qk_slots[m]
            pnm, ps = psum_pair[m % 2]
            w0 = M_(W_(dsnap, "dma"), W_(guard[pnm], "act"))
            for half in range(2):
                for kt in range(KT):
                    mm(ps[:, half * 512:(half + 1) * 512],
                       wsl[:, s, kt * 128:(kt + 1) * 128],
                       hT[:, kt, half * 512:(half + 1) * 512],
                       start=(kt == 0), stop=(kt == KT - 1),
                       wait=M_(w0, gate_hT[kt]))
            slot_last_use[s] = P.snap()
            act(big[:, m, :], ps[:, :], AF.Identity, b=bias[:, m:m + 1],
                wait=bw)
            guard[pnm] = P.snap()
        qk_done = P.snap()

        # ---------------- v (token-major, ones column already in place)
        v_slots = [slot_load(l, Wv[l, kt]) for kt in range(KT)]
        v_dma = M_(*[W_(d, "dma") for _, d in v_slots])
        for m in range(MT_TOK):
            pnm, ps = psum_pair[m % 2]
            w0 = M_(v_dma, W_(guard[pnm], "act"))
            for c0, c1 in ((0, 512), (512, 768)):
                for kt in range(KT):
                    mm(ps[:, c0:c1], hT[:, kt, m * 128:(m + 1) * 128],
                       wsl[:, v_slots[kt][0], c0:c1],
                       start=(kt == 0), stop=(kt == KT - 1),
                       wait=M_(w0, gate_hT[kt]))
            act(vbuf[:, m, :, 0:DH],
                ps[:, 0:H].rearrange("p (h d) -> p h d", d=DH),
                AF.Identity)
            guard[pnm] = P.snap()
        v_use = P.snap()
        for s, _ in v_slots:
            slot_last_use[s] = v_use

        # ---------------- attention: pairs (hp, s); z lands in psum row 64
        # scores pair p -> psA/psB alternating; ctx+z pair -> psS/psT with
        # column halves, so sentences (s, s+2) of one head pair share a psum
        # tensor and z drains in a single [1,1024] copy.
        zf4 = zflat[0:1, :].rearrange("p (h k r t) -> p h k r t",
                                      h=NH, k=2, r=2, t=S)
        ctx_tile_snaps = []
        for hp in range(6):
            for sidx in range(BL):
                p = hp * BL + sidx
                spn, sps = psum_pair[p % 2]
                cpn, cps = ("psS", psS) if p % 2 == 0 else ("psT", psT)
                chalf = (p % 4) // 2
                eslot = p % 2
                # scores: 2 heads at partition rows 0/64 run concurrently
                wsc = M_(W_(guard[spn], "act"), W_(qk_done, "act"))
                for j in range(2):
                    hh = 2 * hp + j
                    prow = 64 * (hh % 2)
                    qt = big[prow:prow + 64, hh // 2, sidx * S:(sidx + 1) * S]
                    ktap = big[prow:prow + 64, 6 + hh // 2,
                               sidx * S:(sidx + 1) * S]
                    for i in range(2):
                        mm(sps[:, (2 * j + i) * S:(2 * j + i + 1) * S],
                           ktap[:, i * 128:(i + 1) * 128], qt,
                           start=True, stop=True, wait=wsc)
                sc_snap = P.snap()
                act(expS[:, eslot, :, :],
                    sps[:, :].rearrange("p (a t) -> p a t", t=S),
                    AF.Exp, scale=1.0 / 8.0, wait=W_(sc_snap, "pe"))
                exp_snap = P.snap()
                guard[spn] = exp_snap
                # ctx (+z in row 64): accumulate over the 2 token tiles
                wctx = M_(W_(exp_snap, "act"), W_(guard[cpn], "dve", "act"))
                for j in range(2):
                    hh = 2 * hp + j
                    for i in range(2):
                        mm(cps[0:DH + 1,
                               chalf * 512 + j * S:chalf * 512 + (j + 1) * S],
                           vbuf[:, 2 * sidx + i, hh, 0:DH + 1],
                           expS[:, eslot, 2 * j + i, :],
                           start=(i == 0), stop=(i == 1), wait=wctx)
                ctx_snap = P.snap()
                # drain ctx rows 0:64 -> ctxT (per head, partition shift)
                for j in range(2):
                    dve_copy(ctxT[64 * j:64 * (j + 1), hp,
                                  sidx * S:(sidx + 1) * S],
                             cps[0:DH, chalf * 512 + j * S:
                                 chalf * 512 + (j + 1) * S],
                             wait=W_(ctx_snap, "pe", "act"))
                # drain z row 64 (both column halves) once both are filled
                if sidx >= 2:
                    r = p % 2
                    dve_copy(zf4[0:1, 2 * hp:2 * hp + 2, :, r:r + 1, :]
                             .rearrange("p a k o t -> p a k (o t)"),
                             cps[DH:DH + 1, :]
                             .rearrange("p (k j t) -> p j k t", k=2, j=2),
                             wait=W_(ctx_snap, "pe", "act"))
                guard[cpn] = P.snap()
            ctx_tile_snaps.append(P.snap())   # ctxT tile hp fully drained
        attn_done = P.snap()

        # ---------------- z transpose; 1/z via exp(-ln(z)) (DVE reciprocal
        # measures ~6.5us/op); keep-warm matmuls stop the PE HAM re-throttle
        dma(zscr[0:1, :], zflat[0:1, :], "dmaZ", wait=W_(attn_done, "dve"))
        zst = P.snap()
        dma(zbufT[:, :], zscr[0:1, :].rearrange("o (h t) -> (o h) t", t=TOK),
            "dmaZ", wait=W_(zst, "dma"))
        zld = P.snap()
        act(zlnf[:, :], zbufT[:, :], AF.Ln, wait=W_(zld, "dma"))
        zl1 = P.snap()
        warm_mm(W_(zl1, "act"))
        act(zbufT[:, :], zlnf[:, :], AF.Exp, scale=-1.0, deps=("act",))
        zrec = P.snap()
        # pass 1: broadcast 1/z and multiply (per-tile dve floor = that
        # tile's attention drains, old by now -> no stall)
        zmul_snaps = []
        for kt in range(KT):
            pnm, ps = psum_pair[kt % 2]
            wz = M_(W_(zrec, "act"), W_(guard[pnm], "dve", "act"))
            for half in range(2):
                mm(ps[:, half * 512:(half + 1) * 512], map12(kt),
                   zbufT[:, half * 512:(half + 1) * 512],
                   start=True, stop=True, wait=wz)
            zmm = P.snap()
            dve_tt(ctxT[:, kt, :], ctxT[:, kt, :], ps[:, :], ALU.mult,
                   wait=M_(W_(zmm, "pe"), W_(ctx_tile_snaps[kt], "dve")))
            guard[pnm] = P.snap()
            zmul_snaps.append(P.snap())
        # pass 2: add v bias (self-waits reference pass-1 counts, loose)
        gate_ctxT = []
        for kt in range(KT):
            dve_tsadd(ctxT[:, kt, :], ctxT[:, kt, :],
                      bias[:, 12 + kt:13 + kt],
                      wait=W_(zmul_snaps[kt], "dve"))
            gate_ctxT.append(W_(P.snap(), "dve"))

        # ---------------- Wo + residual; LN1 sums interleaved
        for m in range(KT):
            s, dsnap = slot_load(l, Wo4[l, m])
            pnm, ps = psum_pair[m % 2]
            w0 = M_(W_(dsnap, "dma"), W_(guard[pnm], "act"))
            for half in range(2):
                for kt in range(KT):
                    mm(ps[:, half * 512:(half + 1) * 512],
                       wsl[:, s, kt * 128:(kt + 1) * 128],
                       ctxT[:, kt, half * 512:(half + 1) * 512],
                       start=(kt == 0), stop=(kt == KT - 1),
                       wait=M_(w0, gate_ctxT[kt]))
            slot_last_use[s] = P.snap()
            act(h1T[:, m, :], ps[:, :], AF.Identity, b=bias[:, 18 + m:19 + m],
                wait=bw)
            guard[pnm] = P.snap()
            dve_tt(h1T[:, m, :], h1T[:, m, :], hT[:, m, :], ALU.add,
                   deps=("act",))
            res_snap = P.snap()
            act(ff1x[:, m, :], h1T[:, m, :], AF.Square,
                wait=W_(res_snap, "dve"))
            sq_snap = P.snap()
            wsum = M_(W_(res_snap, "dve"),
                      W_(guard["psS"], "act", "dve") if m == 0 else {})
            wsq = M_(W_(sq_snap, "act"),
                     W_(guard["psT"], "act", "dve") if m == 0 else {})
            for half in range(2):
                mm(psS[0:1, half * 512:(half + 1) * 512], ones128,
                   h1T[:, m, half * 512:(half + 1) * 512],
                   start=(m == 0), stop=(m == KT - 1), wait=wsum)
                mm(psT[0:1, half * 512:(half + 1) * 512], ones128,
                   ff1x[:, m, half * 512:(half + 1) * 512],
                   start=(m == 0), stop=(m == KT - 1), wait=wsq)
        sums1 = P.snap()

        # ---------------- LN scalar chain + normalize (shared LN1/LN2)
        def layernorm(x, gcol, bcol, sums_snap):
            act(R32(smean[0:1, :]), psS[0:1, :], AF.Identity, scale=1.0 / H,
                wait=W_(sums_snap, "pe"))
            warm_mm(W_(P.snap(), "act"))
            act(se2[0:1, :], psT[0:1, :], AF.Identity, scale=1.0 / H)
            st1 = P.snap()
            warm_mm(W_(st1, "act"))
            dve_tt(R32(sisd[0:1, :]), smean[0:1, :], smean[0:1, :], ALU.mult,
                   wait=M_(W_(st1, "act"), W_(sums_snap, "dve")))
            dve_tt(se2[0:1, :], se2[0:1, :], sisd[0:1, :], ALU.subtract,
                   deps=("dve",))
            P.emit("dve", lambda: nc.vector.tensor_scalar_add(
                se2[0:1, :], se2[0:1, :], 1e-12), deps=("dve",))
            st2 = P.snap()
            warm_mm(W_(st2, "dve"))
            # 1/sqrt(var+eps) = exp(-0.5*ln(var+eps)); ACT Rsqrt is blocked
            # and DVE reciprocal measures ~6.5us per op. Ln lands in zlnf so
            # every writer of sisd rounds to fp32r (walrus verifier).
            act(zlnf[0:1, :], se2[0:1, :], AF.Ln,
                wait=M_(W_(st2, "dve"), W_(zrec, "act")))
            stq = P.snap()
            warm_mm(W_(stq, "act"))
            act(R32(sisd[0:1, :]), zlnf[0:1, :], AF.Exp, scale=-0.5,
                deps=("act",))
            st3 = P.snap()
            guard["psS"] = st3
            guard["psT"] = st3
            wb = M_(W_(st3, "act", "dve"), W_(guard["psA"], "act", "dve"),
                    W_(guard["psB"], "act", "dve"))
            for half in range(2):
                c0, c1 = half * 512, (half + 1) * 512
                mm(psA[:, c0:c1], onesrow32, smean[0:1, c0:c1],
                   start=True, stop=True, wait=wb, raw32=True)
                mm(psB[:, c0:c1], onesrow32, sisd[0:1, c0:c1],
                   start=True, stop=True, wait=wb, raw32=True)
            bc = P.snap()
            mbuf = expS[:, 0, :, :].rearrange("p a t -> p (a t)")
            ibuf = expS[:, 1, :, :].rearrange("p a t -> p (a t)")
            act(mbuf, psA[:, :], AF.Identity, wait=W_(bc, "pe"))
            act(ibuf, psB[:, :], AF.Identity)
            cp = P.snap()
            guard["psA"] = cp
            guard["psB"] = cp
            # three passes so same-tile self-waits reference loose counts
            sub_snaps, mul_snaps, gates = [], [], []
            for kt in range(KT):
                dve_tt(x[:, kt, :], x[:, kt, :], mbuf, ALU.subtract,
                       wait=M_(W_(cp, "act"), W_(sums_snap, "dve")))
                sub_snaps.append(P.snap())
            for kt in range(KT):
                dve_tt(x[:, kt, :], x[:, kt, :], ibuf, ALU.mult,
                       wait=W_(sub_snaps[kt], "dve"))
                mul_snaps.append(P.snap())
            for kt in range(KT):
                dve_ts(x[:, kt, :], x[:, kt, :],
                       bias[:, gcol + kt:gcol + kt + 1],
                       bias[:, bcol + kt:bcol + kt + 1], ALU.mult, ALU.add,
                       wait=W_(mul_snaps[kt], "dve"))
                gates.append(W_(P.snap(), "dve"))
            return gates

        gate_h1T = layernorm(h1T, 24, 30, sums1)

        # ---------------- FF1 (gelu) -> big[0:12] + ctxT + ff1x
        def fftile(m):
            if m < 12:
                return big[:, m, :]
            if m < 18:
                return ctxT[:, m - 12, :]
            return ff1x[:, m - 18, :]

        ff1_gate = []
        for m in range(24):
            s, dsnap = slot_load(l, W14[l, m])
            pnm, ps = psum_pair[m % 2]
            w0 = M_(W_(dsnap, "dma"), W_(guard[pnm], "act"))
            for half in range(2):
                for kt in range(KT):
                    mm(ps[:, half * 512:(half + 1) * 512],
                       wsl[:, s, kt * 128:(kt + 1) * 128],
                       h1T[:, kt, half * 512:(half + 1) * 512],
                       start=(kt == 0), stop=(kt == KT - 1),
                       wait=M_(w0, gate_h1T[kt]))
            slot_last_use[s] = P.snap()
            act(fftile(m), ps[:, :], GELU, b=bias[:, 36 + m:37 + m],
                wait=bw)
            guard[pnm] = P.snap()
            ff1_gate.append(W_(P.snap(), "act"))

        # ---------------- FF2 + residual -> hT; LN2 sums interleaved
        ff2_dma = [None] * KT
        for m in range(4):
            last = wff2_last[m]
            dma(wff2[:, m, :], W24[l, m], f"dmaF{m}",
                wait=W_(last, "pe") if last else None)
            ff2_dma[m] = P.snap()
        for m in range(KT):
            si = m % 4
            pnm, ps = psum_pair[m % 2]
            w0 = M_(W_(ff2_dma[m], "dma"), W_(guard[pnm], "act"))
            for half in range(2):
                for kt in range(24):
                    mm(ps[:, half * 512:(half + 1) * 512],
                       wff2[:, si, kt * 128:(kt + 1) * 128],
                       fftile(kt)[:, half * 512:(half + 1) * 512],
                       start=(kt == 0), stop=(kt == 23),
                       wait=M_(w0, ff1_gate[kt]))
            wff2_last[si] = P.snap()
            if m + 4 < KT:      # prefetch the slot-reusing tile's weights
                nm = m + 4
                dma(wff2[:, nm % 4, :], W24[l, nm], f"dmaF{nm % 4}",
                    wait=W_(wff2_last[nm % 4], "pe"))
                ff2_dma[nm] = P.snap()
            act(hT[:, m, :], ps[:, :], AF.Identity, b=bias[:, 60 + m:61 + m],
                wait=bw)
            guard[pnm] = P.snap()
            dve_tt(hT[:, m, :], hT[:, m, :], h1T[:, m, :], ALU.add,
                   wait=gate_h1T[m], deps=("act",))
            res_snap = P.snap()
            act(sq2[:, m, :], hT[:, m, :], AF.Square,
                wait=W_(res_snap, "dve"))
            sq_snap = P.snap()
            wsum = M_(W_(res_snap, "dve"),
                      W_(guard["psS"], "act", "dve") if m == 0 else {})
            wsq = M_(W_(sq_snap, "act"),
                     W_(guard["psT"], "act", "dve") if m == 0 else {})
            for half in range(2):
                mm(psS[0:1, half * 512:(half + 1) * 512], ones128,
                   hT[:, m, half * 512:(half + 1) * 512],
                   start=(m == 0), stop=(m == KT - 1), wait=wsum)
                mm(psT[0:1, half * 512:(half + 1) * 512], ones128,
                   sq2[:, m, half * 512:(half + 1) * 512],
                   start=(m == 0), stop=(m == KT - 1), wait=wsq)
        sums2 = P.snap()

        gate_hT = layernorm(hT, 66, 72, sums2)
        bias_w = W_(P.snap(), "act", "dve")

    fin = M_(*gate_hT)
    dma(hTout.rearrange("(n p) t -> p n t", p=128), hT[:, :, :], "dmaB",
        wait=fin)

    # ---- replay into engine blocks
    sems = {}
    for name in sorted(P.sem_names):
        sems[name] = ctx.enter_context(nc.semaphore(name))
    with nc.Block() as block:
        @block.tensor
        def _(eng):
            P.replay("pe", eng, sems)

        @block.scalar
        def _(eng):
            P.replay("act", eng, sems)

        @block.vector
        def _(eng):
            P.replay("dve", eng, sems)

        @block.sync
        def _(eng):
            P.replay("dma", eng, sems)

    return nc, ctx


def _pack_consts_bf():
    import ml_dtypes
    c = np.zeros((128, 1024), np.float32)
    c[:, 0] = 1.0                         # ones128
    c[:, 1:1 + MT_TOK * NH] = 1.0         # v ones-column fill source
    for kt in range(KT):
        for f in range(128):
            hh = (kt * 128 + f) // DH
            c[hh, 130 + kt * 128 + f] = 1.0
    return c.astype(ml_dtypes.bfloat16)


def _pack_consts_f32():
    c = np.zeros((128, 128), np.float32)
    c[0, :] = 1.0                         # onesrow32
    return c


def _pack_bias(a):
    out = np.zeros((NL, 128, 80), np.float32)

    def col(vec):                       # feature vec [n*128] -> [128, n]
        return vec.reshape(-1, 128).T

    for l in range(NL):
        out[l, :, 0:18] = col(a['bqkv'][l])
        out[l, :, 18:24] = col(a['bo'][l])
        out[l, :, 24:30] = col(a['ln1_g'][l])
        out[l, :, 30:36] = col(a['ln1_b'][l])
        out[l, :, 36:60] = col(a['b1'][l])
        out[l, :, 60:66] = col(a['b2'][l])
        out[l, :, 66:72] = col(a['ln2_g'][l])
        out[l, :, 72:78] = col(a['ln2_b'][l])
    return out


def _pack_weights(a):
    """Pre-pack weights into m-tile-contiguous bf16 slabs:
    slab[l, m, r, kt*128+c] = W[l, kt*128+r, m*128+c]."""
    import ml_dtypes
    bf = ml_dtypes.bfloat16

    def slab(w, n_in, n_out):
        ktn, mtn = n_in // 128, n_out // 128
        return np.ascontiguousarray(
            np.asarray(w).reshape(NL, ktn, 128, mtn, 128)
            .transpose(0, 3, 2, 1, 4).reshape(NL, mtn, 128, ktn * 128)
        ).astype(bf)

    Wqk = slab(np.ascontiguousarray(a['Wqkv'][:, :, :12 * 128]), H, 12 * 128)
    Wv = np.ascontiguousarray(
        a['Wqkv'][:, :, 12 * 128:18 * 128].reshape(NL, KT, 128, H)).astype(bf)
    Wo4 = slab(a['Wo'], H, H)
    W14 = slab(a['W1'], H, FF)
    W24 = slab(a['W2'], FF, H)
    return {"Wqk": Wqk, "Wv": Wv, "Wo4": Wo4, "W14": W14, "W24": W24}


def _profile_ntff(nc, run_fn):
    """Re-run `run_fn` under NRT/NTFF profiling (core 0) and return
    (results, exec_time_ns, trace_path); (results, None, None) if the
    profiling stack is unavailable. neuron-profile measures only the NEFF
    execution on the device, so the returned time is pure HW exec time."""
    import ctypes
    import tempfile

    try:
        lib = ctypes.CDLL("/opt/axon/libaxon_pjrt.so")
        if not hasattr(lib, "axon_start_nrt_profile"):
            return run_fn(), None, None
    except OSError:
        return run_fn(), None, None
    lib.axon_start_nrt_profile.argtypes = [ctypes.POINTER(ctypes.c_int64),
                                           ctypes.c_size_t]
    lib.axon_start_nrt_profile.restype = ctypes.c_int64
    lib.axon_stop_nrt_profile.argtypes = [ctypes.c_char_p]
    lib.axon_stop_nrt_profile.restype = ctypes.c_int64

    import jax
    jax.devices()
    neff_dir = tempfile.mkdtemp(prefix="bassprof_")
    ids = (ctypes.c_int64 * 1)(0)
    if lib.axon_start_nrt_profile(ids, 1) != 0:
        return run_fn(), None, None
    try:
        results = run_fn()
    finally:
        nfiles = lib.axon_stop_nrt_profile(neff_dir.encode())
    if nfiles <= 0:
        return results, None, None
    try:
        from concourse._compat import FishPath
        import gauge.profiler
        profile = gauge.profiler.Profile(
            profile_path=FishPath(neff_dir),
            kernel_dev_mode=True,
            profile_on_exit=False,
            bass_kernel=nc.m,
            offline_processing=True,
            fname="*_body*",
        )
        pres = profile.to_perfetto(model_index=(0,))
        if pres and pres[0].exec_time_ns:
            return results, int(pres[0].exec_time_ns), pres[0].trace_path
    except Exception as e:
        print("[kernel] ntff processing failed:", e)
    return results, None, None


def run_device(h0, a):
    global LAST_HW_NS
    if not USE_DEVICE:
        return _bert_host(h0, a)
    import time
    import ml_dtypes
    from concourse.bass_utils import run_bass_kernel_spmd

    key = ("enc", DEV_LAYERS)
    if key not in _CACHE:
        _CACHE[key] = _build_encoder(DEV_LAYERS)
    nc, _ctx = _CACHE[key]

    shared = dict(_pack_weights(a))
    shared["biasall"] = _pack_bias(a)
    shared["cbfd"] = _pack_consts_bf()
    shared["cf32d"] = _pack_consts_f32()
    in_maps = []
    for c in range(NCORES):
        hc = h0[c * BL:(c + 1) * BL].reshape(TOK, H).T  # [H, TOK]
        in_maps.append(dict(shared, hT0=np.ascontiguousarray(hc)
                            .astype(ml_dtypes.bfloat16)))

    cores = list(range(NCORES))
    t0 = time.time()
    res = run_bass_kernel_spmd(nc, in_maps, cores)  # compile + warm run
    warm_wall_ns = int((time.time() - t0) * 1e9)
    LAST_HW_NS = warm_wall_ns
    if getattr(res, "exec_time_ns", None):
        LAST_HW_NS = int(res.exec_time_ns)

    if os.environ.get("KERNEL_NO_PROFILE", "") == "":
        try:
            res2, exec_ns, trace = _profile_ntff(
                nc, lambda: run_bass_kernel_spmd(nc, in_maps, cores))
            if exec_ns:
                res = res2
                LAST_HW_NS = exec_ns
                print("[kernel] profile exec_time_ns:", exec_ns,
                      "trace:", trace)
        except Exception as e:
            print("[kernel] profiling failed, using wall time:", e)

    h = np.zeros((B, S, H), np.float32)
    for c in range(NCORES):
        h[c * BL:(c + 1) * BL] = (res.results[c]["hTout"].astype(np.float32)
                                  .T.reshape(BL, S, H))
    if DEV_LAYERS < NL:                 # debugging path: finish on host
        h = _bert_host(h, a, NL - DEV_LAYERS, l0=DEV_LAYERS)
    return h


def kernel(input_ids, attention_mask, labels, emb_tok, emb_pos, emb_type,
           ln_emb_g, ln_emb_b, Wqkv, bqkv, Wo, bo, ln1_g, ln1_b, W1, b1,
           W2, b2, ln2_g, ln2_b, Wih_f, Whh_f, bih_f, bhh_f, Wih_b, Whh_b,
           bih_b, bhh_b, Wc, bc, tag_weight, crf_start, crf_end, crf_trans):
    args = {k: np.asarray(v) for k, v in locals().items()}
    maskf = args['attention_mask'].astype(np.float32)

    h0 = (args['emb_tok'][args['input_ids']] + args['emb_pos'][:S][None]
          + args['emb_type'][0][None, None]).astype(np.float32)
    h0 = _ln_np(h0, args['ln_emb_g'], args['ln_emb_b'])

    h = run_device(h0, args)

    hf = _lstm_host(h, args['Wih_f'], args['Whh_f'], args['bih_f'], args['bhh_f'], False)
    hb = _lstm_host(h, args['Wih_b'], args['Whh_b'], args['bih_b'], args['bhh_b'], True)
    logits = (np.concatenate([hf, hb], -1) @ args['Wc'] + args['bc']) * args['tag_weight']
    ll = _crf_host(logits, args['labels'], maskf, args['crf_start'],
                   args['crf_end'], args['crf_trans'])
    return np.float32(ll.mean())
